# revision 12
# baseline (speedup 1.0000x reference)
"""Trainium2 Bass kernel: 4-layer sliding-window causal transformer (B=2, S=2048,
D=768, H=12, FF=3072, window=128) on 8 NeuronCores.

Sharding: 2 batch groups x 4-way sequence split. Core c handles batch b=c//4,
owning 512 tokens: A=[256q,256q+256) and B=[1024+256q,1024+256q+256) (q=c%4).
The B half covers all possibly-invalid rows (lengths >= S/2); those rows get
dense attention over all 2048 keys (matching the reference's uniform -10000
additive mask, which cancels in softmax), merged by a select mask.

Per layer, TWO AllGathers (bf16) within each 4-core group: kT first (starts
right after the K projection) and V second, so band-local attention and the
Q/V projections overlap the collectives, and remote-key score work starts as
soon as kT lands.  Halo key/value blocks are fetched with register-offset DMAs
whose offsets arrive as per-core input data (keeps the SPMD program uniform).

Everything on-chip is feature-major ([d on partitions, tokens free]); matmuls
run in bf16 with f32 accumulation; LayerNorm/softmax math in f32.
"""
import contextlib
import numpy as np
import ml_dtypes

import concourse.bass as bass
import concourse.tile as tile
from concourse import mybir
from concourse.vector_clock import ScopedClock
from concourse.tile_rust import add_dep_helper
from concourse.bass_utils import run_bass_kernel_spmd

F32 = mybir.dt.float32
BF16 = mybir.dt.bfloat16
U8 = mybir.dt.uint8
I32 = mybir.dt.int32
AF = mybir.ActivationFunctionType
ALU = mybir.AluOpType

B, S, D, H, L, FF, WINDOW, HD = 2, 2048, 768, 12, 4, 3072, 128, 64
C = D // 128          # 6 feature chunks
T = 512               # tokens per core
FFC = FF // 128       # 24
AGW = D * T           # 393216 elements in each AG payload (kT or V)
GROUPS = [[0, 1, 2, 3], [4, 5, 6, 7]]
N_CORES = 8
EPS = 1e-6
SCALE = 1.0 / np.sqrt(HD)


def _rows_for(q):
    return list(range(256 * q, 256 * q + 256)) + list(range(1024 + 256 * q, 1024 + 256 * q + 256))


class TC(tile.TileContext):
    """TileContext workarounds for this toolchain's walrus sync-wait limit:
    at most ONE sync wait per instruction; excess waits move onto injected
    no-fuse NoOps running just before it on the same engine."""
    MAX_WAITS = 1

    def _split_waits(self, insts):
        out = []
        for inst in insts:
            si = getattr(inst, "sync_info", None)
            waits = list(si.on_wait) if si is not None else []
            if len(waits) > self.MAX_WAITS and inst.engine != mybir.EngineType.Unassigned:
                keep = waits[-self.MAX_WAITS:]
                moved = waits[:-self.MAX_WAITS]
                inst.sync_info = mybir.SyncInfo(on_wait=keep, on_update=list(si.on_update))
                for i in range(0, len(moved), self.MAX_WAITS):
                    out.append(mybir.InstNoOp(
                        name=f"{inst.name}-waitnop-{i}",
                        engine=inst.engine,
                        sync_info=mybir.SyncInfo(
                            on_wait=moved[i:i + self.MAX_WAITS], on_update=[]),
                        bass_nofuse=True,
                    ))
            out.append(inst)
        return out

    def _lower_ordered_insts(self, ordered):
        for bb_name in list(ordered.keys()):
            ordered[bb_name] = self._split_waits(ordered[bb_name])
        return super()._lower_ordered_insts(ordered)

    def _drain_and_barrier(self, tick_clock, wait_clock):
        drain_inst = self.nc.sync.drain()
        wait_clock.add_sem_waits(drain_inst.ins, ScopedClock({None: tick_clock.global_clock}))
        si = drain_inst.ins.sync_info
        if si is not None and len(si.on_wait) > self.MAX_WAITS:
            waits = list(si.on_wait)
            drain_inst.ins.sync_info = mybir.SyncInfo(
                on_wait=waits[: self.MAX_WAITS], on_update=list(si.on_update))
            rest = waits[self.MAX_WAITS:]
            for i in range(0, len(rest), self.MAX_WAITS):
                extra = self.nc.sync.drain()
                extra.ins.sync_info = mybir.SyncInfo(
                    on_wait=rest[i: i + self.MAX_WAITS], on_update=[])
        self.nc.all_engine_barrier()
        assert self.sems is not None
        popped = self.nc._tile_sem_poison_stack.pop()
        assert popped is self._sem_poison
        self.nc.clear_and_free_semaphores(list(self.sems.allocated().values()))
        self.nc.all_engine_barrier()


def build_nc(n_layers=L):
    nc = bass.Bass(num_devices=N_CORES)

    # ---------------- I/O ----------------
    xT = nc.dram_tensor("xT", [128, C, T], F32, kind="ExternalInput")
    posT = nc.dram_tensor("posT", [128, C, T], F32, kind="ExternalInput")
    tokf = nc.dram_tensor("tokf", [128, C], F32, kind="ExternalInput")
    eg = nc.dram_tensor("eg", [128, C], F32, kind="ExternalInput")
    ebi = nc.dram_tensor("ebi", [128, C], F32, kind="ExternalInput")
    lng = nc.dram_tensor("lng", [128, L, 2, C], F32, kind="ExternalInput")
    lnb = nc.dram_tensor("lnb", [128, L, 2, C], F32, kind="ExternalInput")
    qbi = nc.dram_tensor("qbi", [128, L, C], F32, kind="ExternalInput")
    obi = nc.dram_tensor("obi", [128, L, C], F32, kind="ExternalInput")
    f1bi = nc.dram_tensor("f1bi", [128, L, FFC], F32, kind="ExternalInput")
    f2bi = nc.dram_tensor("f2bi", [128, L, C], F32, kind="ExternalInput")
    mbandi = nc.dram_tensor("mbandi", [128, 1024], BF16, kind="ExternalInput")
    mseli = nc.dram_tensor("mseli", [128, 256], U8, kind="ExternalInput")
    hoffi = nc.dram_tensor("hoffi", [1, 4], I32, kind="ExternalInput")
    wq = nc.dram_tensor("wq", [L, D, D], BF16, kind="ExternalInput")
    wk = nc.dram_tensor("wk", [L, D, D], BF16, kind="ExternalInput")
    wv = nc.dram_tensor("wv", [L, D, D], BF16, kind="ExternalInput")
    wo = nc.dram_tensor("wo", [L, D, D], BF16, kind="ExternalInput")
    w1 = nc.dram_tensor("w1", [L, D, FF], BF16, kind="ExternalInput")
    w2 = nc.dram_tensor("w2", [L, FF, D], BF16, kind="ExternalInput")
    out_d = nc.dram_tensor("out", [128, C, T], F32, kind="ExternalOutput")

    agiKs = [nc.dram_tensor(f"agiK{l}", [AGW], BF16) for l in range(n_layers)]
    agoKs = [nc.dram_tensor(f"agoK{l}", [4, AGW], BF16) for l in range(n_layers)]
    agiVs = [nc.dram_tensor(f"agiV{l}", [AGW], BF16) for l in range(n_layers)]
    agoVs = [nc.dram_tensor(f"agoV{l}", [4, AGW], BF16) for l in range(n_layers)]

    ln_ctr = [0]

    with contextlib.ExitStack() as ctx:
        tc = ctx.enter_context(TC(nc))
        # ---------------- persistent pools ----------------
        cst = ctx.enter_context(tc.tile_pool(name="cst", bufs=1))
        hpool = ctx.enter_context(tc.tile_pool(name="hp", bufs=1))
        big = ctx.enter_context(tc.tile_pool(name="big", bufs=1))
        sgl = ctx.enter_context(tc.tile_pool(name="sgl", bufs=1))
        scr4 = ctx.enter_context(tc.tile_pool(name="scr4", bufs=4))
        scr2 = ctx.enter_context(tc.tile_pool(name="scr2", bufs=2))
        scr3 = ctx.enter_context(tc.tile_pool(name="scr3", bufs=2))
        wp4 = ctx.enter_context(tc.tile_pool(name="wp4", bufs=7))
        wp1 = ctx.enter_context(tc.tile_pool(name="wp1", bufs=6))
        wp2 = ctx.enter_context(tc.tile_pool(name="wp2", bufs=2))
        ebp = ctx.enter_context(tc.tile_pool(name="ebp", bufs=12))
        edp = ctx.enter_context(tc.tile_pool(name="edp", bufs=4))

        regKA = ctx.enter_context(nc.gpsimd.register("regKA"))
        regKB = ctx.enter_context(nc.gpsimd.register("regKB"))
        regVA = ctx.enter_context(nc.gpsimd.register("regVA"))
        regVB = ctx.enter_context(nc.gpsimd.register("regVB"))

        # ---------------- load constants ----------------
        def cload(nm, shape, dt, src):
            t = cst.tile(shape, dt, tag=nm, name=nm)
            nc.sync.dma_start(out=t, in_=src)
            return t

        tok_s = cload("tok_s", [128, C], F32, tokf[:, :])
        eg_s = cload("eg_s", [128, C], F32, eg[:, :])
        eb_s = cload("eb_s", [128, C], F32, ebi[:, :])
        lng_s = cload("lng_s", [128, L, 2, C], F32, lng[:, :, :, :])
        lnb_s = cload("lnb_s", [128, L, 2, C], F32, lnb[:, :, :, :])
        qb_s = cload("qb_s", [128, L, C], F32, qbi[:, :, :])
        ob_s = cload("ob_s", [128, L, C], F32, obi[:, :, :])
        f1b_s = cload("f1b_s", [128, L, FFC], F32, f1bi[:, :, :])
        f2b_s = cload("f2b_s", [128, L, C], F32, f2bi[:, :, :])
        mband = cload("mband", [128, 1024], BF16, mbandi[:, :])
        msel = cload("msel", [128, 256], U8, mseli[:, :])
        hoff_s = cload("hoff_s", [1, 4], I32, hoffi[:, :])

        onesP = cst.tile([128, 1], F32)
        nc.vector.memset(onesP, 1.0)
        onesPb = cst.tile([128, 1], BF16)
        nc.vector.memset(onesPb, 1.0)
        onesPP = cst.tile([128, 128], F32)
        nc.vector.memset(onesPP, 1.0)
        eps_row = cst.tile([1, 1], F32)
        nc.vector.memset(eps_row, EPS)

        nc.gpsimd.reg_load(regKA, hoff_s[0:1, 0:1])
        nc.gpsimd.reg_load(regKB, hoff_s[0:1, 1:2])
        nc.gpsimd.reg_load(regVA, hoff_s[0:1, 2:3])
        nc.gpsimd.reg_load(regVB, hoff_s[0:1, 3:4])

        # ---------------- persistent tiles (hoisted out of the layer loop) ----
        h = hpool.tile([128, C, T], F32)
        hb = sgl.tile([128, C, T], BF16, tag="hb")
        qT = sgl.tile([128, C, T], BF16, tag="qT")
        kT = sgl.tile([128, C, T], BF16, tag="kT")
        v65o = sgl.tile([128, 4, H, 65], BF16, tag="v65o")
        kTf = big.tile([128, C, 4, T], BF16, tag="kTf")
        v65f = big.tile([128, 4, 4, H, 65], BF16, tag="v65f")
        kThalo = sgl.tile([128, C, 2, 128], BF16, tag="kThalo")
        v65h = sgl.tile([128, 2, H, 65], BF16, tag="v65h")
        cb_sb = big.tile([128, H, T], BF16, tag="cbs")
        ctxT = sgl.tile([128, C, T], BF16, tag="ctxT")
        recip = sgl.tile([1, H, T], F32, tag="recip")
        nc.vector.memset(v65o[:, :, :, 64:65], 1.0)
        nc.vector.memset(v65f[:, :, :, :, 64:65], 1.0)
        nc.vector.memset(v65h[:, :, :, 64:65], 1.0)

        # views: eb column space is [hA*A0|A0*A0|A0*A1|A1*A1|hB*B0|B0*B0|B0*B1|B1*B1]
        def half_view(t_, lo, hi):
            return t_.rearrange("p (a b) -> p a b", a=2)[:, :, lo:hi]

        # ---------------- h / embeddings ----------------
        nc.sync.dma_start(out=h, in_=xT[:, :, :])
        for k in range(C):
            pk = scr2.tile([128, T], F32, tag="lnt")
            nc.sync.dma_start(out=pk, in_=posT[:, k, :])
            nc.vector.tensor_add(out=h[:, k, :], in0=h[:, k, :], in1=pk)
            nc.vector.tensor_scalar_add(h[:, k, :], h[:, k, :], tok_s[:, k:k + 1])

        def emit_ln(g_ap, b_ap):
            """post-LN over features (partition axis), h updated in place."""
            ln_ctr[0] += 1
            with tc.tile_pool(name=f"lnps{ln_ctr[0]}", bufs=1, space="PSUM") as lnps, \
                 tc.tile_pool(name=f"lnps2_{ln_ctr[0]}", bufs=2, space="PSUM") as lnps2:
                s1 = lnps.tile([1, T], F32, tag="s1")
                s2 = lnps.tile([1, T], F32, tag="s2")
                for k in range(C):
                    sq = scr2.tile([128, T], BF16, tag="sq")
                    nc.vector.tensor_mul(out=sq, in0=h[:, k, :], in1=h[:, k, :])
                    nc.tensor.matmul(s1, onesP[:, :], h[:, k, :],
                                     start=(k == 0), stop=(k == C - 1))
                    nc.tensor.matmul(s2, onesPb[:, :], sq,
                                     start=(k == 0), stop=(k == C - 1))
                mean = scr4.tile([1, T], F32, tag="row")
                nc.scalar.activation(out=mean, in_=s1, func=AF.Copy, scale=1.0 / D)
                ex2 = scr4.tile([1, T], F32, tag="row")
                nc.scalar.activation(out=ex2, in_=s2, func=AF.Copy, scale=1.0 / D)
                var = scr4.tile([1, T], F32, tag="row")
                nc.gpsimd.tensor_mul(out=var, in0=mean, in1=mean)
                nc.gpsimd.tensor_tensor(out=var, in0=ex2, in1=var, op=ALU.subtract)
                nc.scalar.activation(out=var, in_=var, func=AF.Ln,
                                     bias=eps_row[0:1, 0:1])
                rstd = scr4.tile([1, T], F32, tag="row")
                nc.scalar.activation(out=rstd, in_=var, func=AF.Exp, scale=-0.5)
                mb = lnps2.tile([128, T], F32, tag="mr")
                nc.tensor.matmul(mb, onesPP[0:1, :], mean, start=True, stop=True)
                rb = lnps2.tile([128, T], F32, tag="mr")
                nc.tensor.matmul(rb, onesPP[0:1, :], rstd, start=True, stop=True)
                for k in range(C):
                    d_t = scr2.tile([128, T], F32, tag="lnt")
                    nc.vector.tensor_tensor(out=d_t, in0=h[:, k, :], in1=mb,
                                            op=ALU.subtract)
                    nc.vector.tensor_tensor(out=d_t, in0=d_t, in1=rb, op=ALU.mult)
                    nc.vector.tensor_scalar(out=h[:, k, :], in0=d_t,
                                            scalar1=g_ap[:, k:k + 1],
                                            scalar2=b_ap[:, k:k + 1],
                                            op0=ALU.mult, op1=ALU.add)

        emit_ln(eg_s[:, :], eb_s[:, :])

        # ---------------- layers ----------------
        for l in range(n_layers):
            agiK, agoK, agiV, agoV = agiKs[l], agoKs[l], agiVs[l], agoVs[l]

            # bf16 cast of h for matmul inputs
            for k in range(C):
                nc.scalar.activation(out=hb[:, k, :], in_=h[:, k, :], func=AF.Copy)

            with tc.tile_pool(name=f"psqkv{l}", bufs=3, space="PSUM") as psq:
                # ---- K projection first, kick off its AllGather ----
                wk_t = []
                for k in range(C):
                    wt_ = wp4.tile([128, D], BF16, tag="w4")
                    nc.sync.dma_start(out=wt_, in_=wk[l, 128 * k:128 * (k + 1), :])
                    wk_t.append(wt_)
                for m in range(C):
                    ps = psq.tile([128, T], F32, tag="ps")
                    for k in range(C):
                        nc.tensor.matmul(ps, wk_t[k][:, 128 * m:128 * (m + 1)],
                                         hb[:, k, :], start=(k == 0), stop=(k == C - 1))
                    nc.vector.tensor_copy(out=kT[:, m, :], in_=ps)
                wkt = nc.sync.dma_start(
                    out=agiK[:].rearrange("(k p n) -> p k n", k=C, p=128, n=T),
                    in_=kT[:, :, :])
                ccK = nc.gpsimd.collective_compute(
                    "AllGather", ALU.bypass, replica_groups=GROUPS,
                    ins=[agiK[:]], outs=[agoK[:, :]])
                add_dep_helper(ccK.ins, wkt.ins, reason="AG-K waits kT bounce")

                # ---- V projection, kick off its AllGather ----
                wv_t = []
                for k in range(C):
                    wt_ = wp4.tile([128, D], BF16, tag="w4")
                    nc.sync.dma_start(out=wt_, in_=wv[l, 128 * k:128 * (k + 1), :])
                    wv_t.append(wt_)
                for t in range(4):
                    ps = psq.tile([128, T], F32, tag="ps")
                    ps2 = psq.tile([128, 256], F32, tag="ps2")
                    for k in range(C):
                        nc.tensor.matmul(ps, hb[:, k, 128 * t:128 * (t + 1)],
                                         wv_t[k][:, 0:512], start=(k == 0), stop=(k == C - 1))
                        nc.tensor.matmul(ps2, hb[:, k, 128 * t:128 * (t + 1)],
                                         wv_t[k][:, 512:768], start=(k == 0), stop=(k == C - 1))
                    nc.vector.tensor_copy(
                        out=v65o[:, t, 0:8, 0:64],
                        in_=ps[:, :].rearrange("p (h d) -> p h d", h=8))
                    nc.vector.tensor_copy(
                        out=v65o[:, t, 8:12, 0:64],
                        in_=ps2[:, :].rearrange("p (h d) -> p h d", h=4))
                wvts = []
                for t in range(4):
                    wvt = nc.sync.dma_start(
                        out=agiV[t * 128 * D:(t + 1) * 128 * D].rearrange(
                            "(p h d) -> p h d", p=128, h=H, d=64),
                        in_=v65o[:, t, :, 0:64])
                    wvts.append(wvt)
                ccV = nc.gpsimd.collective_compute(
                    "AllGather", ALU.bypass, replica_groups=GROUPS,
                    ins=[agiV[:]], outs=[agoV[:, :]])
                for wvt in wvts:
                    add_dep_helper(ccV.ins, wvt.ins, reason="AG-V waits V bounce")

                # ---- Q projection ----
                wq_t = []
                for k in range(C):
                    wt_ = wp4.tile([128, D], BF16, tag="w4")
                    nc.sync.dma_start(out=wt_, in_=wq[l, 128 * k:128 * (k + 1), :])
                    wq_t.append(wt_)
                for m in range(C):
                    ps = psq.tile([128, T], F32, tag="ps")
                    for k in range(C):
                        nc.tensor.matmul(ps, wq_t[k][:, 128 * m:128 * (m + 1)],
                                         hb[:, k, :], start=(k == 0), stop=(k == C - 1))
                    nc.vector.tensor_scalar_add(qT[:, m, :], ps, qb_s[:, l, m:m + 1])

            # ---- band-local scores + exp (overlaps the AllGathers) ----
            ebs = []
            with tc.tile_pool(name=f"psBL{l}", bufs=2, space="PSUM") as psbl:
                for hd_i in range(H):
                    hp, ck = 64 * (hd_i % 2), hd_i // 2
                    sc = psbl.tile([128, 1024], F32, tag="scL")
                    segs = [
                        (128, 256, kT[hp:hp + 64, ck, 0:128], 0, True, False),
                        (384, 128, kT[hp:hp + 64, ck, 128:256], 128, False, True),
                        (640, 256, kT[hp:hp + 64, ck, 256:384], 256, True, False),
                        (896, 128, kT[hp:hp + 64, ck, 384:512], 384, False, True),
                    ]
                    for cs, w, lhs, qs, st, sp in segs:
                        nc.tensor.matmul(sc[:, cs:cs + w], lhs,
                                         qT[hp:hp + 64, ck, qs:qs + w],
                                         start=st, stop=sp, skip_group_check=True)
                    eb_t = ebp.tile([128, 1024], BF16, tag="expB", name=f"eb_{l}_{hd_i}")
                    nc.scalar.activation(out=half_view(eb_t, 128, 512),
                                         in_=half_view(sc, 128, 512),
                                         func=AF.Exp, scale=SCALE)
                    nc.gpsimd.tensor_mul(out=half_view(eb_t, 128, 512),
                                         in0=half_view(eb_t, 128, 512),
                                         in1=half_view(mband, 128, 512))
                    ebs.append(eb_t)

            # ---- fetch gathered kT + halo keys (waits AG-K) ----
            for r in range(4):
                f1 = nc.sync.dma_start(
                    out=kTf[:, :, r, :],
                    in_=agoK[r, :].rearrange("(k p n) -> p k n", k=C, p=128, n=T))
                add_dep_helper(f1.ins, ccK.ins, reason="kTf fetch waits AG-K")
            for s, rk in enumerate([regKA, regKB]):
                fh = nc.gpsimd.dma_start(
                    out=kThalo[:, :, s, :],
                    in_=bass.AP(agoK, rk, [[T, 128], [128 * T, C], [1, 128]]))
                add_dep_helper(fh.ins, ccK.ins, reason="kT halo waits AG-K")

            # ---- band halo scores + exp ----
            with tc.tile_pool(name=f"psBH{l}", bufs=2, space="PSUM") as psbh:
                for hd_i in range(H):
                    hp, ck = 64 * (hd_i % 2), hd_i // 2
                    sch = psbh.tile([128, 256], F32, tag="scH")
                    nc.tensor.matmul(sch[:, 0:128], kThalo[hp:hp + 64, ck, 0, :],
                                     qT[hp:hp + 64, ck, 0:128],
                                     start=True, stop=False, skip_group_check=True)
                    nc.tensor.matmul(sch[:, 128:256], kThalo[hp:hp + 64, ck, 1, :],
                                     qT[hp:hp + 64, ck, 256:384],
                                     start=False, stop=True, skip_group_check=True)
                    eb_t = ebs[hd_i]
                    nc.scalar.activation(out=half_view(eb_t, 0, 128),
                                         in_=sch.rearrange("p (a b) -> p a b", a=2),
                                         func=AF.Exp, scale=SCALE)
                    nc.gpsimd.tensor_mul(out=half_view(eb_t, 0, 128),
                                         in0=half_view(eb_t, 0, 128),
                                         in1=half_view(mband, 0, 128))

            # ---- fetch gathered V + halo values (waits AG-V) ----
            for r in range(4):
                for t in range(4):
                    f2 = nc.sync.dma_start(
                        out=v65f[:, r, t, :, 0:64],
                        in_=agoV[r, 128 * D * t:128 * D * (t + 1)].rearrange(
                            "(p h d) -> p h d", p=128, h=H, d=64))
                    add_dep_helper(f2.ins, ccV.ins, reason="V fetch waits AG-V")
            for s, rv in enumerate([regVA, regVB]):
                fv = nc.gpsimd.dma_start(
                    out=v65h[:, s, :, 0:64],
                    in_=bass.AP(agoV, rv, [[D, 128], [64, H], [1, 64]]))
                add_dep_helper(fv.ins, ccV.ins, reason="V halo waits AG-V")

            # ---- dense attention for the B half: r-outer, AV consumed eagerly ----
            with tc.tile_pool(name=f"psDc{l}", bufs=3, space="PSUM") as psdc:
                for half in range(2):
                    hds = list(range(6 * half, 6 * half + 6))
                    cdps = [psdc.tile([128, 512], F32, tag="ctxd",
                                      name=f"cdp_{l}_{half}_{i}") for i in range(3)]
                    with tc.tile_pool(name=f"psD{l}_{half}", bufs=2, space="PSUM") as psd:
                        for r in range(4):
                            for sub in range(2):
                                shds = hds[3 * sub:3 * sub + 3]
                                eds = []
                                for hd_i in shds:
                                    hp, ck = 64 * (hd_i % 2), hd_i // 2
                                    sd = psd.tile([128, 1024], F32, tag="scD")
                                    for j in range(4):
                                        nc.tensor.matmul(
                                            sd[:, 256 * j:256 * (j + 1)],
                                            kTf[hp:hp + 64, ck, r, 128 * j:128 * (j + 1)],
                                            qT[hp:hp + 64, ck, 256:512],
                                            start=(j % 2 == 0), stop=(j % 2 == 1),
                                            skip_group_check=True)
                                    ed = edp.tile([128, 1024], BF16, tag="expD")
                                    nc.scalar.activation(out=ed, in_=sd, func=AF.Exp,
                                                         scale=SCALE)
                                    eds.append(ed)
                                for i, hd_i in enumerate(shds):
                                    cd = cdps[(hd_i % 6) // 2]
                                    co = 256 * (hd_i % 2)
                                    for j in range(4):
                                        nc.tensor.matmul(
                                            cd[0:65, co:co + 256], v65f[:, r, j, hd_i, :],
                                            eds[i][:, 256 * j:256 * (j + 1)],
                                            start=(r == 0 and j == 0 and hd_i % 2 == 0),
                                            stop=(r == 3 and j == 3 and hd_i % 2 == 1),
                                            skip_group_check=True)
                    # band AV + blend for this half's heads
                    with tc.tile_pool(name=f"psA{l}_{half}", bufs=2, space="PSUM") as psa:
                        for hd_i in hds:
                            cps = psa.tile([128, T], F32, tag="ctx")
                            av = [
                                (0, v65h[:, 0, hd_i, :], 0),
                                (0, v65o[:, 0, hd_i, :], 128),
                                (128, v65o[:, 0, hd_i, :], 256),
                                (128, v65o[:, 1, hd_i, :], 384),
                                (256, v65h[:, 1, hd_i, :], 512),
                                (256, v65o[:, 2, hd_i, :], 640),
                                (384, v65o[:, 2, hd_i, :], 768),
                                (384, v65o[:, 3, hd_i, :], 896),
                            ]
                            eb_t = ebs[hd_i]
                            for i, (ocs, vsrc, ecs) in enumerate(av):
                                nc.tensor.matmul(cps[0:65, ocs:ocs + 128], vsrc,
                                                 eb_t[:, ecs:ecs + 128],
                                                 start=(i == 0), stop=(i == len(av) - 1),
                                                 skip_group_check=True)
                            nc.vector.tensor_copy(out=cb_sb[0:65, hd_i, :],
                                                  in_=cps[0:65, :])
                            cd = cdps[(hd_i % 6) // 2]
                            co = 256 * (hd_i % 2)
                            nc.vector.copy_predicated(
                                cb_sb[0:65, hd_i, 256:512], msel[0:65, :],
                                cd[0:65, co:co + 256])

            # ---- normalize ctx, pack to feature-major ctxT ----
            nc.vector.reciprocal(out=recip[0:1, :, :], in_=cb_sb[64:65, :, :])
            with tc.tile_pool(name=f"psN{l}", bufs=2, space="PSUM") as psn:
                for c in range(C):
                    he, ho = 2 * c, 2 * c + 1
                    rbe = psn.tile([128, T], F32, tag="rps")
                    nc.tensor.matmul(rbe[0:64, :], onesPP[0:1, 0:64], recip[0:1, he, :],
                                     start=True, stop=True)
                    rbo = psn.tile([128, T], F32, tag="rps")
                    nc.tensor.matmul(rbo[0:64, :], onesPP[0:1, 0:64], recip[0:1, ho, :],
                                     start=True, stop=True)
                    nc.vector.tensor_mul(out=ctxT[0:64, c, :],
                                         in0=cb_sb[0:64, he, :], in1=rbe[0:64, :])
                    nc.vector.tensor_mul(out=ctxT[64:128, c, :],
                                         in0=cb_sb[0:64, ho, :], in1=rbo[0:64, :])

            # ---- O projection + residual + LN ----
            with tc.tile_pool(name=f"psO{l}", bufs=2, space="PSUM") as pso:
                wo_t = []
                for k in range(C):
                    wt_ = wp4.tile([128, D], BF16, tag="w4")
                    nc.sync.dma_start(out=wt_, in_=wo[l, 128 * k:128 * (k + 1), :])
                    wo_t.append(wt_)
                for m in range(C):
                    ps = pso.tile([128, T], F32, tag="ops")
                    for k in range(C):
                        nc.tensor.matmul(ps, wo_t[k][:, 128 * m:128 * (m + 1)],
                                         ctxT[:, k, :], start=(k == 0), stop=(k == C - 1))
                    nc.vector.scalar_tensor_tensor(
                        out=h[:, m, :], in0=ps, scalar=ob_s[:, l, m:m + 1],
                        in1=h[:, m, :], op0=ALU.add, op1=ALU.add)
            emit_ln(lng_s[:, l, 0, :], lnb_s[:, l, 0, :])

            # ---- FFN + residual + LN ----
            for k in range(C):
                nc.scalar.activation(out=hb[:, k, :], in_=h[:, k, :], func=AF.Copy)
            with tc.tile_pool(name=f"psF1{l}", bufs=2, space="PSUM") as psf1, \
                 tc.tile_pool(name=f"psF2{l}", bufs=6, space="PSUM") as psf2:
                f2ps = [psf2.tile([128, T], F32, tag="f2", name=f"f2ps_{l}_{m}") for m in range(C)]
                for quarter in range(4):
                    w1_t = []
                    for k in range(C):
                        wt_ = wp1.tile([128, FF // 4], BF16, tag="w1t")
                        nc.sync.dma_start(
                            out=wt_, in_=w1[l, 128 * k:128 * (k + 1),
                                            quarter * (FF // 4):(quarter + 1) * (FF // 4)])
                        w1_t.append(wt_)
                    for fi in range(FFC // 4):
                        f = quarter * (FFC // 4) + fi
                        w2_t = wp2.tile([128, D], BF16, tag="w2t")
                        nc.sync.dma_start(out=w2_t, in_=w2[l, 128 * f:128 * (f + 1), :])
                        ps1 = psf1.tile([128, T], F32, tag="f1")
                        for k in range(C):
                            nc.tensor.matmul(ps1, w1_t[k][:, 128 * fi:128 * (fi + 1)],
                                             hb[:, k, :], start=(k == 0), stop=(k == C - 1))
                        rl = scr3.tile([128, T], BF16, tag="rl")
                        nc.scalar.activation(out=rl, in_=ps1, func=AF.Relu,
                                             bias=f1b_s[:, l, f:f + 1])
                        for m in range(C):
                            nc.tensor.matmul(f2ps[m], w2_t[:, 128 * m:128 * (m + 1)], rl,
                                             start=(f == 0), stop=(f == FFC - 1))
                for m in range(C):
                    nc.vector.scalar_tensor_tensor(
                        out=h[:, m, :], in0=f2ps[m], scalar=f2b_s[:, l, m:m + 1],
                        in1=h[:, m, :], op0=ALU.add, op1=ALU.add)
            emit_ln(lng_s[:, l, 1, :], lnb_s[:, l, 1, :])

        nc.sync.dma_start(out=out_d[:, :, :], in_=h)
    return nc


def _feat_pack(v):
    """[768] -> [128, 6] feature-major"""
    return np.ascontiguousarray(np.asarray(v, np.float32).reshape(C, 128).T)


def _tok_pack(arr):
    """[512, 768] -> [128, 6, 512] feature-major"""
    a = np.asarray(arr, np.float32).T.reshape(C, 128, T)
    return np.ascontiguousarray(a.transpose(1, 0, 2))


def make_in_maps(inputs):
    lengths = np.asarray(inputs["lengths"]).astype(np.int64)
    window = int(np.asarray(inputs["window"]))
    assert window == WINDOW and np.all(lengths >= S // 2)
    x = np.asarray(inputs["inputs"], np.float32)
    pos = np.asarray(inputs["pos_emb"], np.float32)
    tok = np.asarray(inputs["tok_emb"], np.float32)

    bf = ml_dtypes.bfloat16
    ow = np.asarray(inputs["o_w"], np.float32)
    vb = np.asarray(inputs["v_b"], np.float32)
    ob_eff = np.asarray(inputs["o_b"], np.float32) + np.einsum("ld,lde->le", vb, ow)

    shared = {
        "tokf": _feat_pack(tok[0]),
        "eg": _feat_pack(inputs["emb_ln_g"]),
        "ebi": _feat_pack(inputs["emb_ln_b"]),
        "lng": np.ascontiguousarray(np.stack(
            [np.stack([_feat_pack(np.asarray(inputs["attn_ln_g"])[li]),
                       _feat_pack(np.asarray(inputs["out_ln_g"])[li])], 1)
             for li in range(L)], 1), np.float32),
        "lnb": np.ascontiguousarray(np.stack(
            [np.stack([_feat_pack(np.asarray(inputs["attn_ln_b"])[li]),
                       _feat_pack(np.asarray(inputs["out_ln_b"])[li])], 1)
             for li in range(L)], 1), np.float32),
        "qbi": np.ascontiguousarray(np.stack(
            [_feat_pack(np.asarray(inputs["q_b"])[li]) for li in range(L)], 1), np.float32),
        "obi": np.ascontiguousarray(np.stack(
            [_feat_pack(ob_eff[li]) for li in range(L)], 1), np.float32),
        "f1bi": np.ascontiguousarray(np.stack(
            [np.ascontiguousarray(np.asarray(inputs["ff1_b"], np.float32)[li].reshape(FFC, 128).T)
             for li in range(L)], 1), np.float32),
        "f2bi": np.ascontiguousarray(np.stack(
            [_feat_pack(np.asarray(inputs["ff2_b"])[li]) for li in range(L)], 1), np.float32),
        "wq": np.asarray(inputs["q_w"], np.float32).astype(bf),
        "wk": np.asarray(inputs["k_w"], np.float32).astype(bf),
        "wv": np.asarray(inputs["v_w"], np.float32).astype(bf),
        "wo": np.asarray(inputs["o_w"], np.float32).astype(bf),
        "w1": np.asarray(inputs["ff1_w"], np.float32).astype(bf),
        "w2": np.asarray(inputs["ff2_w"], np.float32).astype(bf),
    }

    io = np.arange(128)
    m_own = (io[:, None] <= io[None, :]).astype(np.float32)      # key off <= q off
    m_prev = (io[:, None] > io[None, :]).astype(np.float32)      # prev-block band
    in_maps = []
    for cidx in range(N_CORES):
        b, q = cidx // 4, cidx % 4
        rows = _rows_for(q)
        lb_ = int(lengths[b])
        mhA = np.zeros((128, 128), np.float32) if q == 0 else m_prev
        # segments: [haloA | b0:own+prev | b1:own | haloB:prev | b2:own+prev | b3:own]
        mb_ = np.concatenate([mhA, m_own, m_prev, m_own,
                              m_prev, m_own, m_prev, m_own], axis=1)
        msel_ = (np.asarray([1024 + 256 * q + cc_ for cc_ in range(256)]) >= lb_)
        gA = 2 * q - 1 if q > 0 else 15
        gB = 7 + 2 * q

        def _owner(g):
            return g // 2 if g < 8 else (g - 8) // 2

        def _lb(g):
            return g % 2 if g < 8 else 2 + (g - 8) % 2

        hoff = []
        for g in (gA, gB):
            hoff.append(_owner(g) * AGW + 128 * _lb(g))          # kT halo in agoK
        for g in (gA, gB):
            hoff.append(_owner(g) * AGW + 128 * _lb(g) * D)      # V halo in agoV
        im = dict(shared)
        im["xT"] = _tok_pack(x[b][rows])
        im["posT"] = _tok_pack(pos[rows])
        im["mbandi"] = mb_.astype(bf)
        im["mseli"] = np.broadcast_to(msel_, (128, 256)).astype(np.uint8).copy()
        im["hoffi"] = np.asarray([hoff], np.int32)
        in_maps.append(im)
    return in_maps


def assemble_output(results):
    out = np.zeros((B, S, D), np.float32)
    for cidx in range(N_CORES):
        b, q = cidx // 4, cidx % 4
        rows = _rows_for(q)
        hc = np.asarray(results[cidx]["out"])          # [128, 6, 512]
        out[b, rows, :] = hc.transpose(1, 0, 2).reshape(D, T).T
    return out


_cached = {}


def kernel(**inputs):
    n_layers = int(inputs.pop("_n_layers", L))
    if n_layers not in _cached:
        _cached[n_layers] = build_nc(n_layers)
    nc = _cached[n_layers]
    in_maps = make_in_maps(inputs)
    res = run_bass_kernel_spmd(nc, in_maps, core_ids=list(range(N_CORES)))
    return assemble_output(res.results)


# revision 13
# speedup vs baseline: 1.0267x; 1.0267x over previous
"""Trainium2 Bass kernel: 4-layer sliding-window causal transformer (B=2, S=2048,
D=768, H=12, FF=3072, window=128) on 8 NeuronCores.

Sharding: 2 batch groups x 4-way sequence split. Core c handles batch b=c//4,
owning 512 tokens: A=[256q,256q+256) and B=[1024+256q,1024+256q+256) (q=c%4).
The B half covers all possibly-invalid rows (lengths >= S/2); those rows get
dense attention over all 2048 keys (matching the reference's uniform -10000
additive mask, which cancels in softmax), merged by a select mask.

Per layer, TWO AllGathers (bf16) within each 4-core group: kT first (starts
right after the K projection) and V second, so band-local attention and the
Q/V projections overlap the collectives, and remote-key score work starts as
soon as kT lands.  Halo key/value blocks are fetched with register-offset DMAs
whose offsets arrive as per-core input data (keeps the SPMD program uniform).

Everything on-chip is feature-major ([d on partitions, tokens free]); matmuls
run in bf16 with f32 accumulation; LayerNorm/softmax math in f32.
"""
import contextlib
import numpy as np
import ml_dtypes

import concourse.bass as bass
import concourse.tile as tile
from concourse import mybir
from concourse.vector_clock import ScopedClock
from concourse.tile_rust import add_dep_helper
from concourse.bass_utils import run_bass_kernel_spmd

F32 = mybir.dt.float32
BF16 = mybir.dt.bfloat16
U8 = mybir.dt.uint8
I32 = mybir.dt.int32
AF = mybir.ActivationFunctionType
ALU = mybir.AluOpType

B, S, D, H, L, FF, WINDOW, HD = 2, 2048, 768, 12, 4, 3072, 128, 64
C = D // 128          # 6 feature chunks
T = 512               # tokens per core
FFC = FF // 128       # 24
AGW = D * T           # 393216 elements in the kT AG payload
AGWV = 4 * H * 65 * 128  # V AG payload: per-partition [t, h, 65] incl. ones column
GROUPS = [[0, 1, 2, 3], [4, 5, 6, 7]]
N_CORES = 8
EPS = 1e-6
SCALE = 1.0 / np.sqrt(HD)


def _rows_for(q):
    return list(range(256 * q, 256 * q + 256)) + list(range(1024 + 256 * q, 1024 + 256 * q + 256))


class TC(tile.TileContext):
    """TileContext workarounds for this toolchain's walrus sync-wait limit:
    at most ONE sync wait per instruction; excess waits move onto injected
    no-fuse NoOps running just before it on the same engine."""
    MAX_WAITS = 1

    def _split_waits(self, insts):
        out = []
        for inst in insts:
            si = getattr(inst, "sync_info", None)
            waits = list(si.on_wait) if si is not None else []
            if len(waits) > self.MAX_WAITS and inst.engine != mybir.EngineType.Unassigned:
                keep = waits[-self.MAX_WAITS:]
                moved = waits[:-self.MAX_WAITS]
                inst.sync_info = mybir.SyncInfo(on_wait=keep, on_update=list(si.on_update))
                for i in range(0, len(moved), self.MAX_WAITS):
                    out.append(mybir.InstNoOp(
                        name=f"{inst.name}-waitnop-{i}",
                        engine=inst.engine,
                        sync_info=mybir.SyncInfo(
                            on_wait=moved[i:i + self.MAX_WAITS], on_update=[]),
                        bass_nofuse=True,
                    ))
            out.append(inst)
        return out

    def _lower_ordered_insts(self, ordered):
        for bb_name in list(ordered.keys()):
            ordered[bb_name] = self._split_waits(ordered[bb_name])
        return super()._lower_ordered_insts(ordered)

    def _drain_and_barrier(self, tick_clock, wait_clock):
        drain_inst = self.nc.sync.drain()
        wait_clock.add_sem_waits(drain_inst.ins, ScopedClock({None: tick_clock.global_clock}))
        si = drain_inst.ins.sync_info
        if si is not None and len(si.on_wait) > self.MAX_WAITS:
            waits = list(si.on_wait)
            drain_inst.ins.sync_info = mybir.SyncInfo(
                on_wait=waits[: self.MAX_WAITS], on_update=list(si.on_update))
            rest = waits[self.MAX_WAITS:]
            for i in range(0, len(rest), self.MAX_WAITS):
                extra = self.nc.sync.drain()
                extra.ins.sync_info = mybir.SyncInfo(
                    on_wait=rest[i: i + self.MAX_WAITS], on_update=[])
        self.nc.all_engine_barrier()
        assert self.sems is not None
        popped = self.nc._tile_sem_poison_stack.pop()
        assert popped is self._sem_poison
        self.nc.clear_and_free_semaphores(list(self.sems.allocated().values()))
        self.nc.all_engine_barrier()


def build_nc(n_layers=L):
    nc = bass.Bass(num_devices=N_CORES)

    # ---------------- I/O ----------------
    xT = nc.dram_tensor("xT", [128, C, T], F32, kind="ExternalInput")
    posT = nc.dram_tensor("posT", [128, C, T], F32, kind="ExternalInput")
    tokf = nc.dram_tensor("tokf", [128, C], F32, kind="ExternalInput")
    eg = nc.dram_tensor("eg", [128, C], F32, kind="ExternalInput")
    ebi = nc.dram_tensor("ebi", [128, C], F32, kind="ExternalInput")
    lng = nc.dram_tensor("lng", [128, L, 2, C], F32, kind="ExternalInput")
    lnb = nc.dram_tensor("lnb", [128, L, 2, C], F32, kind="ExternalInput")
    qbi = nc.dram_tensor("qbi", [128, L, C], F32, kind="ExternalInput")
    obi = nc.dram_tensor("obi", [128, L, C], F32, kind="ExternalInput")
    f1bi = nc.dram_tensor("f1bi", [128, L, FFC], F32, kind="ExternalInput")
    f2bi = nc.dram_tensor("f2bi", [128, L, C], F32, kind="ExternalInput")
    mbandi = nc.dram_tensor("mbandi", [128, 1024], BF16, kind="ExternalInput")
    mseli = nc.dram_tensor("mseli", [128, 256], U8, kind="ExternalInput")
    hoffi = nc.dram_tensor("hoffi", [1, 4], I32, kind="ExternalInput")
    wq = nc.dram_tensor("wq", [L, D, D], BF16, kind="ExternalInput")
    wk = nc.dram_tensor("wk", [L, D, D], BF16, kind="ExternalInput")
    wv = nc.dram_tensor("wv", [L, D, D], BF16, kind="ExternalInput")
    wo = nc.dram_tensor("wo", [L, D, D], BF16, kind="ExternalInput")
    w1 = nc.dram_tensor("w1", [L, D, FF], BF16, kind="ExternalInput")
    w2 = nc.dram_tensor("w2", [L, FF, D], BF16, kind="ExternalInput")
    out_d = nc.dram_tensor("out", [128, C, T], F32, kind="ExternalOutput")

    agiKs = [nc.dram_tensor(f"agiK{l}", [AGW], BF16) for l in range(n_layers)]
    agoKs = [nc.dram_tensor(f"agoK{l}", [4, AGW], BF16) for l in range(n_layers)]
    agiVs = [nc.dram_tensor(f"agiV{l}", [AGWV], BF16) for l in range(n_layers)]
    agoVs = [nc.dram_tensor(f"agoV{l}", [4, AGWV], BF16) for l in range(n_layers)]

    ln_ctr = [0]

    with contextlib.ExitStack() as ctx:
        tc = ctx.enter_context(TC(nc))
        # ---------------- persistent pools ----------------
        cst = ctx.enter_context(tc.tile_pool(name="cst", bufs=1))
        hpool = ctx.enter_context(tc.tile_pool(name="hp", bufs=1))
        big = ctx.enter_context(tc.tile_pool(name="big", bufs=1))
        sgl = ctx.enter_context(tc.tile_pool(name="sgl", bufs=1))
        scr4 = ctx.enter_context(tc.tile_pool(name="scr4", bufs=4))
        scr2 = ctx.enter_context(tc.tile_pool(name="scr2", bufs=2))
        scr3 = ctx.enter_context(tc.tile_pool(name="scr3", bufs=2))
        wp4 = ctx.enter_context(tc.tile_pool(name="wp4", bufs=7))
        wp1 = ctx.enter_context(tc.tile_pool(name="wp1", bufs=6))
        wp2 = ctx.enter_context(tc.tile_pool(name="wp2", bufs=2))
        ebp = ctx.enter_context(tc.tile_pool(name="ebp", bufs=12))
        edp = ctx.enter_context(tc.tile_pool(name="edp", bufs=4))

        regKA = ctx.enter_context(nc.gpsimd.register("regKA"))
        regKB = ctx.enter_context(nc.gpsimd.register("regKB"))
        regVA = ctx.enter_context(nc.gpsimd.register("regVA"))
        regVB = ctx.enter_context(nc.gpsimd.register("regVB"))

        # ---------------- load constants ----------------
        def cload(nm, shape, dt, src):
            t = cst.tile(shape, dt, tag=nm, name=nm)
            nc.sync.dma_start(out=t, in_=src)
            return t

        tok_s = cload("tok_s", [128, C], F32, tokf[:, :])
        eg_s = cload("eg_s", [128, C], F32, eg[:, :])
        eb_s = cload("eb_s", [128, C], F32, ebi[:, :])
        lng_s = cload("lng_s", [128, L, 2, C], F32, lng[:, :, :, :])
        lnb_s = cload("lnb_s", [128, L, 2, C], F32, lnb[:, :, :, :])
        qb_s = cload("qb_s", [128, L, C], F32, qbi[:, :, :])
        ob_s = cload("ob_s", [128, L, C], F32, obi[:, :, :])
        f1b_s = cload("f1b_s", [128, L, FFC], F32, f1bi[:, :, :])
        f2b_s = cload("f2b_s", [128, L, C], F32, f2bi[:, :, :])
        mband = cload("mband", [128, 1024], BF16, mbandi[:, :])
        msel = cload("msel", [128, 256], U8, mseli[:, :])
        hoff_s = cload("hoff_s", [1, 4], I32, hoffi[:, :])

        onesP = cst.tile([128, 1], F32)
        nc.vector.memset(onesP, 1.0)
        onesPb = cst.tile([128, 1], BF16)
        nc.vector.memset(onesPb, 1.0)
        onesPP = cst.tile([128, 128], F32)
        nc.vector.memset(onesPP, 1.0)
        eps_row = cst.tile([1, 1], F32)
        nc.vector.memset(eps_row, EPS)

        nc.gpsimd.reg_load(regKA, hoff_s[0:1, 0:1])
        nc.gpsimd.reg_load(regKB, hoff_s[0:1, 1:2])
        nc.gpsimd.reg_load(regVA, hoff_s[0:1, 2:3])
        nc.gpsimd.reg_load(regVB, hoff_s[0:1, 3:4])

        # ---------------- persistent tiles (hoisted out of the layer loop) ----
        h = hpool.tile([128, C, T], F32)
        hb = sgl.tile([128, C, T], BF16, tag="hb")
        qT = sgl.tile([128, C, T], BF16, tag="qT")
        kT = sgl.tile([128, C, T], BF16, tag="kT")
        v65o = sgl.tile([128, 4, H, 65], BF16, tag="v65o")
        kTf = big.tile([128, 4, C, T], BF16, tag="kTf")
        v65f = big.tile([128, 4, 4, H, 65], BF16, tag="v65f")
        kThalo = sgl.tile([128, C, 2, 128], BF16, tag="kThalo")
        v65h = sgl.tile([128, 2, H, 65], BF16, tag="v65h")
        cb_sb = big.tile([128, H, T], BF16, tag="cbs")
        ctxT = sgl.tile([128, C, T], BF16, tag="ctxT")
        recip = sgl.tile([1, H, T], F32, tag="recip")
        nc.vector.memset(v65o[:, :, :, 64:65], 1.0)

        # views: eb column space is [hA*A0|A0*A0|A0*A1|A1*A1|hB*B0|B0*B0|B0*B1|B1*B1]
        def half_view(t_, lo, hi):
            return t_.rearrange("p (a b) -> p a b", a=2)[:, :, lo:hi]

        # ---------------- h / embeddings ----------------
        nc.sync.dma_start(out=h, in_=xT[:, :, :])
        for k in range(C):
            pk = scr2.tile([128, T], F32, tag="lnt")
            nc.sync.dma_start(out=pk, in_=posT[:, k, :])
            nc.vector.tensor_add(out=h[:, k, :], in0=h[:, k, :], in1=pk)
            nc.vector.tensor_scalar_add(h[:, k, :], h[:, k, :], tok_s[:, k:k + 1])

        def emit_ln(g_ap, b_ap):
            """post-LN over features (partition axis), h updated in place."""
            ln_ctr[0] += 1
            with tc.tile_pool(name=f"lnps{ln_ctr[0]}", bufs=1, space="PSUM") as lnps, \
                 tc.tile_pool(name=f"lnps2_{ln_ctr[0]}", bufs=2, space="PSUM") as lnps2:
                s1 = lnps.tile([1, T], F32, tag="s1")
                s2 = lnps.tile([1, T], F32, tag="s2")
                for k in range(C):
                    sq = scr2.tile([128, T], BF16, tag="sq")
                    nc.vector.tensor_mul(out=sq, in0=h[:, k, :], in1=h[:, k, :])
                    nc.tensor.matmul(s1, onesP[:, :], h[:, k, :],
                                     start=(k == 0), stop=(k == C - 1))
                    nc.tensor.matmul(s2, onesPb[:, :], sq,
                                     start=(k == 0), stop=(k == C - 1))
                mean = scr4.tile([1, T], F32, tag="row")
                nc.scalar.activation(out=mean, in_=s1, func=AF.Copy, scale=1.0 / D)
                ex2 = scr4.tile([1, T], F32, tag="row")
                nc.scalar.activation(out=ex2, in_=s2, func=AF.Copy, scale=1.0 / D)
                var = scr4.tile([1, T], F32, tag="row")
                nc.gpsimd.tensor_mul(out=var, in0=mean, in1=mean)
                nc.gpsimd.tensor_tensor(out=var, in0=ex2, in1=var, op=ALU.subtract)
                nc.scalar.activation(out=var, in_=var, func=AF.Ln,
                                     bias=eps_row[0:1, 0:1])
                rstd = scr4.tile([1, T], F32, tag="row")
                nc.scalar.activation(out=rstd, in_=var, func=AF.Exp, scale=-0.5)
                mb = lnps2.tile([128, T], F32, tag="mr")
                nc.tensor.matmul(mb, onesPP[0:1, :], mean, start=True, stop=True)
                rb = lnps2.tile([128, T], F32, tag="mr")
                nc.tensor.matmul(rb, onesPP[0:1, :], rstd, start=True, stop=True)
                for k in range(C):
                    d_t = scr2.tile([128, T], F32, tag="lnt")
                    nc.vector.tensor_tensor(out=d_t, in0=h[:, k, :], in1=mb,
                                            op=ALU.subtract)
                    nc.vector.tensor_tensor(out=d_t, in0=d_t, in1=rb, op=ALU.mult)
                    nc.vector.tensor_scalar(out=h[:, k, :], in0=d_t,
                                            scalar1=g_ap[:, k:k + 1],
                                            scalar2=b_ap[:, k:k + 1],
                                            op0=ALU.mult, op1=ALU.add)

        emit_ln(eg_s[:, :], eb_s[:, :])

        # ---------------- layers ----------------
        wk_next = [None]
        for l in range(n_layers):
            agiK, agoK, agiV, agoV = agiKs[l], agoKs[l], agiVs[l], agoVs[l]

            # bf16 cast of h for matmul inputs
            for k in range(C):
                nc.scalar.activation(out=hb[:, k, :], in_=h[:, k, :], func=AF.Copy)

            with tc.tile_pool(name=f"psqkv{l}", bufs=3, space="PSUM") as psq:
                # ---- K projection first, kick off its AllGather ----
                # (wk tiles were prefetched: layer 0 below, layers 1+ during the
                #  previous layer's O-projection section)
                if l == 0:
                    wk_t = []
                    for k in range(C):
                        wt_ = wp4.tile([128, D], BF16, tag="w4")
                        nc.sync.dma_start(out=wt_, in_=wk[l, 128 * k:128 * (k + 1), :])
                        wk_t.append(wt_)
                else:
                    wk_t = wk_next[0]
                for m in range(C):
                    ps = psq.tile([128, T], F32, tag="ps")
                    for k in range(C):
                        nc.tensor.matmul(ps, wk_t[k][:, 128 * m:128 * (m + 1)],
                                         hb[:, k, :], start=(k == 0), stop=(k == C - 1))
                    nc.vector.tensor_copy(out=kT[:, m, :], in_=ps)
                wkt = nc.sync.dma_start(
                    out=agiK[:].rearrange("(p k n) -> p k n", p=128, k=C, n=T),
                    in_=kT[:, :, :])
                ccK = nc.gpsimd.collective_compute(
                    "AllGather", ALU.bypass, replica_groups=GROUPS,
                    ins=[agiK[:]], outs=[agoK[:, :]])
                add_dep_helper(ccK.ins, wkt.ins, reason="AG-K waits kT bounce")

                # ---- V projection, kick off its AllGather ----
                wv_t = []
                for k in range(C):
                    wt_ = wp4.tile([128, D], BF16, tag="w4")
                    nc.sync.dma_start(out=wt_, in_=wv[l, 128 * k:128 * (k + 1), :])
                    wv_t.append(wt_)
                for t in range(4):
                    ps = psq.tile([128, T], F32, tag="ps")
                    ps2 = psq.tile([128, 256], F32, tag="ps2")
                    for k in range(C):
                        nc.tensor.matmul(ps, hb[:, k, 128 * t:128 * (t + 1)],
                                         wv_t[k][:, 0:512], start=(k == 0), stop=(k == C - 1))
                        nc.tensor.matmul(ps2, hb[:, k, 128 * t:128 * (t + 1)],
                                         wv_t[k][:, 512:768], start=(k == 0), stop=(k == C - 1))
                    nc.vector.tensor_copy(
                        out=v65o[:, t, 0:8, 0:64],
                        in_=ps[:, :].rearrange("p (h d) -> p h d", h=8))
                    nc.vector.tensor_copy(
                        out=v65o[:, t, 8:12, 0:64],
                        in_=ps2[:, :].rearrange("p (h d) -> p h d", h=4))
                wvt = nc.sync.dma_start(
                    out=agiV[:].rearrange("(p x) -> p x", p=128, x=4 * H * 65),
                    in_=v65o[:, :, :, :])
                ccV = nc.gpsimd.collective_compute(
                    "AllGather", ALU.bypass, replica_groups=GROUPS,
                    ins=[agiV[:]], outs=[agoV[:, :]])
                add_dep_helper(ccV.ins, wvt.ins, reason="AG-V waits V bounce")

                # ---- Q projection ----
                wq_t = []
                for k in range(C):
                    wt_ = wp4.tile([128, D], BF16, tag="w4")
                    nc.sync.dma_start(out=wt_, in_=wq[l, 128 * k:128 * (k + 1), :])
                    wq_t.append(wt_)
                for m in range(C):
                    ps = psq.tile([128, T], F32, tag="ps")
                    for k in range(C):
                        nc.tensor.matmul(ps, wq_t[k][:, 128 * m:128 * (m + 1)],
                                         hb[:, k, :], start=(k == 0), stop=(k == C - 1))
                    nc.vector.tensor_scalar_add(qT[:, m, :], ps, qb_s[:, l, m:m + 1])

            # ---- band-local scores + exp (overlaps the AllGathers) ----
            ebs = []
            with tc.tile_pool(name=f"psBL{l}", bufs=2, space="PSUM") as psbl:
                for hd_i in range(H):
                    hp, ck = 64 * (hd_i % 2), hd_i // 2
                    sc = psbl.tile([128, 1024], F32, tag="scL")
                    segs = [
                        (128, 256, kT[hp:hp + 64, ck, 0:128], 0, True, False),
                        (384, 128, kT[hp:hp + 64, ck, 128:256], 128, False, True),
                        (640, 256, kT[hp:hp + 64, ck, 256:384], 256, True, False),
                        (896, 128, kT[hp:hp + 64, ck, 384:512], 384, False, True),
                    ]
                    for cs, w, lhs, qs, st, sp in segs:
                        nc.tensor.matmul(sc[:, cs:cs + w], lhs,
                                         qT[hp:hp + 64, ck, qs:qs + w],
                                         start=st, stop=sp, skip_group_check=True)
                    eb_t = ebp.tile([128, 1024], BF16, tag="expB", name=f"eb_{l}_{hd_i}")
                    nc.scalar.activation(out=half_view(eb_t, 128, 512),
                                         in_=half_view(sc, 128, 512),
                                         func=AF.Exp, scale=SCALE)
                    nc.gpsimd.tensor_mul(out=half_view(eb_t, 128, 512),
                                         in0=half_view(eb_t, 128, 512),
                                         in1=half_view(mband, 128, 512))
                    ebs.append(eb_t)

            # ---- fetch gathered kT + halo keys (waits AG-K) ----
            for r in range(4):
                f1 = nc.sync.dma_start(
                    out=kTf[:, r, :, :],
                    in_=agoK[r, :].rearrange("(p k n) -> p k n", p=128, k=C, n=T))
                add_dep_helper(f1.ins, ccK.ins, reason="kTf fetch waits AG-K")
            for s, rk in enumerate([regKA, regKB]):
                fh = nc.gpsimd.dma_start(
                    out=kThalo[:, :, s, :],
                    in_=bass.AP(agoK, rk, [[C * T, 128], [T, C], [1, 128]]))
                add_dep_helper(fh.ins, ccK.ins, reason="kT halo waits AG-K")

            # ---- band halo scores + exp ----
            with tc.tile_pool(name=f"psBH{l}", bufs=2, space="PSUM") as psbh:
                for hd_i in range(H):
                    hp, ck = 64 * (hd_i % 2), hd_i // 2
                    sch = psbh.tile([128, 256], F32, tag="scH")
                    nc.tensor.matmul(sch[:, 0:128], kThalo[hp:hp + 64, ck, 0, :],
                                     qT[hp:hp + 64, ck, 0:128],
                                     start=True, stop=False, skip_group_check=True)
                    nc.tensor.matmul(sch[:, 128:256], kThalo[hp:hp + 64, ck, 1, :],
                                     qT[hp:hp + 64, ck, 256:384],
                                     start=False, stop=True, skip_group_check=True)
                    eb_t = ebs[hd_i]
                    nc.scalar.activation(out=half_view(eb_t, 0, 128),
                                         in_=sch.rearrange("p (a b) -> p a b", a=2),
                                         func=AF.Exp, scale=SCALE)
                    nc.gpsimd.tensor_mul(out=half_view(eb_t, 0, 128),
                                         in0=half_view(eb_t, 0, 128),
                                         in1=half_view(mband, 0, 128))

            # ---- fetch gathered V + halo values (waits AG-V) ----
            for r in range(4):
                f2 = nc.sync.dma_start(
                    out=v65f[:, r, :, :, :],
                    in_=agoV[r, :].rearrange("(p t h e) -> p t h e",
                                             p=128, t=4, h=H, e=65))
                add_dep_helper(f2.ins, ccV.ins, reason="V fetch waits AG-V")
            for s, rv in enumerate([regVA, regVB]):
                fv = nc.gpsimd.dma_start(
                    out=v65h[:, s, :, :],
                    in_=bass.AP(agoV, rv, [[4 * H * 65, 128], [1, H * 65]]))
                add_dep_helper(fv.ins, ccV.ins, reason="V halo waits AG-V")

            # ---- dense attention for the B half: r-outer, AV consumed eagerly ----
            with tc.tile_pool(name=f"psDc{l}", bufs=3, space="PSUM") as psdc:
                for half in range(2):
                    hds = list(range(6 * half, 6 * half + 6))
                    cdps = [psdc.tile([128, 512], F32, tag="ctxd",
                                      name=f"cdp_{l}_{half}_{i}") for i in range(3)]
                    with tc.tile_pool(name=f"psD{l}_{half}", bufs=2, space="PSUM") as psd:
                        for r in range(4):
                            for sub in range(2):
                                shds = hds[3 * sub:3 * sub + 3]
                                eds = []
                                for hd_i in shds:
                                    hp, ck = 64 * (hd_i % 2), hd_i // 2
                                    sd = psd.tile([128, 1024], F32, tag="scD")
                                    for j in range(4):
                                        nc.tensor.matmul(
                                            sd[:, 256 * j:256 * (j + 1)],
                                            kTf[hp:hp + 64, r, ck, 128 * j:128 * (j + 1)],
                                            qT[hp:hp + 64, ck, 256:512],
                                            start=(j % 2 == 0), stop=(j % 2 == 1),
                                            skip_group_check=True)
                                    ed = edp.tile([128, 1024], BF16, tag="expD")
                                    nc.scalar.activation(out=ed, in_=sd, func=AF.Exp,
                                                         scale=SCALE)
                                    eds.append(ed)
                                for i, hd_i in enumerate(shds):
                                    cd = cdps[(hd_i % 6) // 2]
                                    co = 256 * (hd_i % 2)
                                    for j in range(4):
                                        nc.tensor.matmul(
                                            cd[0:65, co:co + 256], v65f[:, r, j, hd_i, :],
                                            eds[i][:, 256 * j:256 * (j + 1)],
                                            start=(r == 0 and j == 0 and hd_i % 2 == 0),
                                            stop=(r == 3 and j == 3 and hd_i % 2 == 1),
                                            skip_group_check=True)
                    # band AV + blend for this half's heads
                    with tc.tile_pool(name=f"psA{l}_{half}", bufs=2, space="PSUM") as psa:
                        for hd_i in hds:
                            cps = psa.tile([128, T], F32, tag="ctx")
                            av = [
                                (0, v65h[:, 0, hd_i, :], 0),
                                (0, v65o[:, 0, hd_i, :], 128),
                                (128, v65o[:, 0, hd_i, :], 256),
                                (128, v65o[:, 1, hd_i, :], 384),
                                (256, v65h[:, 1, hd_i, :], 512),
                                (256, v65o[:, 2, hd_i, :], 640),
                                (384, v65o[:, 2, hd_i, :], 768),
                                (384, v65o[:, 3, hd_i, :], 896),
                            ]
                            eb_t = ebs[hd_i]
                            for i, (ocs, vsrc, ecs) in enumerate(av):
                                nc.tensor.matmul(cps[0:65, ocs:ocs + 128], vsrc,
                                                 eb_t[:, ecs:ecs + 128],
                                                 start=(i == 0), stop=(i == len(av) - 1),
                                                 skip_group_check=True)
                            nc.vector.tensor_copy(out=cb_sb[0:65, hd_i, :],
                                                  in_=cps[0:65, :])
                            cd = cdps[(hd_i % 6) // 2]
                            co = 256 * (hd_i % 2)
                            nc.vector.copy_predicated(
                                cb_sb[0:65, hd_i, 256:512], msel[0:65, :],
                                cd[0:65, co:co + 256])
                            nc.vector.reciprocal(out=recip[0:1, hd_i, :],
                                                 in_=cb_sb[64:65, hd_i, :])

            # ---- normalize ctx, pack to feature-major ctxT ----
            with tc.tile_pool(name=f"psN{l}", bufs=2, space="PSUM") as psn:
                for c in range(C):
                    he, ho = 2 * c, 2 * c + 1
                    rbe = psn.tile([128, T], F32, tag="rps")
                    nc.tensor.matmul(rbe[0:64, :], onesPP[0:1, 0:64], recip[0:1, he, :],
                                     start=True, stop=True)
                    rbo = psn.tile([128, T], F32, tag="rps")
                    nc.tensor.matmul(rbo[0:64, :], onesPP[0:1, 0:64], recip[0:1, ho, :],
                                     start=True, stop=True)
                    nc.vector.tensor_mul(out=ctxT[0:64, c, :],
                                         in0=cb_sb[0:64, he, :], in1=rbe[0:64, :])
                    nc.vector.tensor_mul(out=ctxT[64:128, c, :],
                                         in0=cb_sb[0:64, ho, :], in1=rbo[0:64, :])

            # ---- O projection + residual + LN ----
            with tc.tile_pool(name=f"psO{l}", bufs=2, space="PSUM") as pso:
                wo_t = []
                for k in range(C):
                    wt_ = wp4.tile([128, D], BF16, tag="w4")
                    nc.sync.dma_start(out=wt_, in_=wo[l, 128 * k:128 * (k + 1), :])
                    wo_t.append(wt_)
                if l + 1 < n_layers:
                    wk_next[0] = []
                    for k in range(C):
                        wt_ = wp4.tile([128, D], BF16, tag="w4")
                        nc.sync.dma_start(out=wt_,
                                          in_=wk[l + 1, 128 * k:128 * (k + 1), :])
                        wk_next[0].append(wt_)
                for m in range(C):
                    ps = pso.tile([128, T], F32, tag="ops")
                    for k in range(C):
                        nc.tensor.matmul(ps, wo_t[k][:, 128 * m:128 * (m + 1)],
                                         ctxT[:, k, :], start=(k == 0), stop=(k == C - 1))
                    nc.vector.scalar_tensor_tensor(
                        out=h[:, m, :], in0=ps, scalar=ob_s[:, l, m:m + 1],
                        in1=h[:, m, :], op0=ALU.add, op1=ALU.add)
            emit_ln(lng_s[:, l, 0, :], lnb_s[:, l, 0, :])

            # ---- FFN + residual + LN ----
            for k in range(C):
                nc.scalar.activation(out=hb[:, k, :], in_=h[:, k, :], func=AF.Copy)
            with tc.tile_pool(name=f"psF1{l}", bufs=2, space="PSUM") as psf1, \
                 tc.tile_pool(name=f"psF2{l}", bufs=6, space="PSUM") as psf2:
                f2ps = [psf2.tile([128, T], F32, tag="f2", name=f"f2ps_{l}_{m}") for m in range(C)]
                for quarter in range(4):
                    w1_t = []
                    for k in range(C):
                        wt_ = wp1.tile([128, FF // 4], BF16, tag="w1t")
                        nc.sync.dma_start(
                            out=wt_, in_=w1[l, 128 * k:128 * (k + 1),
                                            quarter * (FF // 4):(quarter + 1) * (FF // 4)])
                        w1_t.append(wt_)
                    for fi in range(FFC // 4):
                        f = quarter * (FFC // 4) + fi
                        w2_t = wp2.tile([128, D], BF16, tag="w2t")
                        nc.sync.dma_start(out=w2_t, in_=w2[l, 128 * f:128 * (f + 1), :])
                        ps1 = psf1.tile([128, T], F32, tag="f1")
                        for k in range(C):
                            nc.tensor.matmul(ps1, w1_t[k][:, 128 * fi:128 * (fi + 1)],
                                             hb[:, k, :], start=(k == 0), stop=(k == C - 1))
                        rl = scr3.tile([128, T], BF16, tag="rl")
                        nc.scalar.activation(out=rl, in_=ps1, func=AF.Relu,
                                             bias=f1b_s[:, l, f:f + 1])
                        for m in range(C):
                            nc.tensor.matmul(f2ps[m], w2_t[:, 128 * m:128 * (m + 1)], rl,
                                             start=(f == 0), stop=(f == FFC - 1))
                for m in range(C):
                    nc.vector.scalar_tensor_tensor(
                        out=h[:, m, :], in0=f2ps[m], scalar=f2b_s[:, l, m:m + 1],
                        in1=h[:, m, :], op0=ALU.add, op1=ALU.add)
            emit_ln(lng_s[:, l, 1, :], lnb_s[:, l, 1, :])

        nc.sync.dma_start(out=out_d[:, :, :], in_=h)
    return nc


def _feat_pack(v):
    """[768] -> [128, 6] feature-major"""
    return np.ascontiguousarray(np.asarray(v, np.float32).reshape(C, 128).T)


def _tok_pack(arr):
    """[512, 768] -> [128, 6, 512] feature-major"""
    a = np.asarray(arr, np.float32).T.reshape(C, 128, T)
    return np.ascontiguousarray(a.transpose(1, 0, 2))


def make_in_maps(inputs):
    lengths = np.asarray(inputs["lengths"]).astype(np.int64)
    window = int(np.asarray(inputs["window"]))
    assert window == WINDOW and np.all(lengths >= S // 2)
    x = np.asarray(inputs["inputs"], np.float32)
    pos = np.asarray(inputs["pos_emb"], np.float32)
    tok = np.asarray(inputs["tok_emb"], np.float32)

    bf = ml_dtypes.bfloat16
    ow = np.asarray(inputs["o_w"], np.float32)
    vb = np.asarray(inputs["v_b"], np.float32)
    ob_eff = np.asarray(inputs["o_b"], np.float32) + np.einsum("ld,lde->le", vb, ow)

    shared = {
        "tokf": _feat_pack(tok[0]),
        "eg": _feat_pack(inputs["emb_ln_g"]),
        "ebi": _feat_pack(inputs["emb_ln_b"]),
        "lng": np.ascontiguousarray(np.stack(
            [np.stack([_feat_pack(np.asarray(inputs["attn_ln_g"])[li]),
                       _feat_pack(np.asarray(inputs["out_ln_g"])[li])], 1)
             for li in range(L)], 1), np.float32),
        "lnb": np.ascontiguousarray(np.stack(
            [np.stack([_feat_pack(np.asarray(inputs["attn_ln_b"])[li]),
                       _feat_pack(np.asarray(inputs["out_ln_b"])[li])], 1)
             for li in range(L)], 1), np.float32),
        "qbi": np.ascontiguousarray(np.stack(
            [_feat_pack(np.asarray(inputs["q_b"])[li]) for li in range(L)], 1), np.float32),
        "obi": np.ascontiguousarray(np.stack(
            [_feat_pack(ob_eff[li]) for li in range(L)], 1), np.float32),
        "f1bi": np.ascontiguousarray(np.stack(
            [np.ascontiguousarray(np.asarray(inputs["ff1_b"], np.float32)[li].reshape(FFC, 128).T)
             for li in range(L)], 1), np.float32),
        "f2bi": np.ascontiguousarray(np.stack(
            [_feat_pack(np.asarray(inputs["ff2_b"])[li]) for li in range(L)], 1), np.float32),
        "wq": np.asarray(inputs["q_w"], np.float32).astype(bf),
        "wk": np.asarray(inputs["k_w"], np.float32).astype(bf),
        "wv": np.asarray(inputs["v_w"], np.float32).astype(bf),
        "wo": np.asarray(inputs["o_w"], np.float32).astype(bf),
        "w1": np.asarray(inputs["ff1_w"], np.float32).astype(bf),
        "w2": np.asarray(inputs["ff2_w"], np.float32).astype(bf),
    }

    io = np.arange(128)
    m_own = (io[:, None] <= io[None, :]).astype(np.float32)      # key off <= q off
    m_prev = (io[:, None] > io[None, :]).astype(np.float32)      # prev-block band
    in_maps = []
    for cidx in range(N_CORES):
        b, q = cidx // 4, cidx % 4
        rows = _rows_for(q)
        lb_ = int(lengths[b])
        mhA = np.zeros((128, 128), np.float32) if q == 0 else m_prev
        # segments: [haloA | b0:own+prev | b1:own | haloB:prev | b2:own+prev | b3:own]
        mb_ = np.concatenate([mhA, m_own, m_prev, m_own,
                              m_prev, m_own, m_prev, m_own], axis=1)
        msel_ = (np.asarray([1024 + 256 * q + cc_ for cc_ in range(256)]) >= lb_)
        gA = 2 * q - 1 if q > 0 else 15
        gB = 7 + 2 * q

        def _owner(g):
            return g // 2 if g < 8 else (g - 8) // 2

        def _lb(g):
            return g % 2 if g < 8 else 2 + (g - 8) % 2

        hoff = []
        for g in (gA, gB):
            hoff.append(_owner(g) * AGW + 128 * _lb(g))          # kT halo in agoK
        for g in (gA, gB):
            hoff.append(_owner(g) * AGWV + _lb(g) * H * 65)      # V halo in agoV
        im = dict(shared)
        im["xT"] = _tok_pack(x[b][rows])
        im["posT"] = _tok_pack(pos[rows])
        im["mbandi"] = mb_.astype(bf)
        im["mseli"] = np.broadcast_to(msel_, (128, 256)).astype(np.uint8).copy()
        im["hoffi"] = np.asarray([hoff], np.int32)
        in_maps.append(im)
    return in_maps


def assemble_output(results):
    out = np.zeros((B, S, D), np.float32)
    for cidx in range(N_CORES):
        b, q = cidx // 4, cidx % 4
        rows = _rows_for(q)
        hc = np.asarray(results[cidx]["out"])          # [128, 6, 512]
        out[b, rows, :] = hc.transpose(1, 0, 2).reshape(D, T).T
    return out


_cached = {}


def kernel(**inputs):
    n_layers = int(inputs.pop("_n_layers", L))
    if n_layers not in _cached:
        _cached[n_layers] = build_nc(n_layers)
    nc = _cached[n_layers]
    in_maps = make_in_maps(inputs)
    res = run_bass_kernel_spmd(nc, in_maps, core_ids=list(range(N_CORES)))
    return assemble_output(res.results)


# revision 14
# speedup vs baseline: 1.0423x; 1.0152x over previous
"""Trainium2 Bass kernel: 4-layer sliding-window causal transformer (B=2, S=2048,
D=768, H=12, FF=3072, window=128) on 8 NeuronCores.

Sharding: 2 batch groups x 4-way sequence split. Core c handles batch b=c//4,
owning 512 tokens: A=[256q,256q+256) and B=[1024+256q,1024+256q+256) (q=c%4).
The B half covers all possibly-invalid rows (lengths >= S/2); those rows get
dense attention over all 2048 keys (matching the reference's uniform -10000
additive mask, which cancels in softmax), merged by a select mask.

Per layer, TWO AllGathers (bf16) within each 4-core group: kT first (starts
right after the K projection) and V second, so band-local attention and the
Q/V projections overlap the collectives, and remote-key score work starts as
soon as kT lands.  Halo key/value blocks are fetched with register-offset DMAs
whose offsets arrive as per-core input data (keeps the SPMD program uniform).

Everything on-chip is feature-major ([d on partitions, tokens free]); matmuls
run in bf16 with f32 accumulation; LayerNorm/softmax math in f32.
"""
import contextlib
import numpy as np
import ml_dtypes

import concourse.bass as bass
import concourse.tile as tile
from concourse import mybir
from concourse.vector_clock import ScopedClock
from concourse.tile_rust import add_dep_helper
from concourse.bass_utils import run_bass_kernel_spmd

F32 = mybir.dt.float32
BF16 = mybir.dt.bfloat16
U8 = mybir.dt.uint8
I32 = mybir.dt.int32
AF = mybir.ActivationFunctionType
ALU = mybir.AluOpType

B, S, D, H, L, FF, WINDOW, HD = 2, 2048, 768, 12, 4, 3072, 128, 64
C = D // 128          # 6 feature chunks
T = 512               # tokens per core
FFC = FF // 128       # 24
AGW = D * T           # 393216 elements in the kT AG payload
AGWV = 4 * H * 65 * 128  # V AG payload: per-partition [t, h, 65] incl. ones column
GROUPS = [[0, 1, 2, 3], [4, 5, 6, 7]]
N_CORES = 8
EPS = 1e-6
SCALE = 1.0 / np.sqrt(HD)


def _rows_for(q):
    return list(range(256 * q, 256 * q + 256)) + list(range(1024 + 256 * q, 1024 + 256 * q + 256))


class TC(tile.TileContext):
    """TileContext workarounds for this toolchain's walrus sync-wait limit:
    at most ONE sync wait per instruction; excess waits move onto injected
    no-fuse NoOps running just before it on the same engine."""
    MAX_WAITS = 1

    def _split_waits(self, insts):
        out = []
        for inst in insts:
            si = getattr(inst, "sync_info", None)
            waits = list(si.on_wait) if si is not None else []
            if len(waits) > self.MAX_WAITS and inst.engine != mybir.EngineType.Unassigned:
                keep = waits[-self.MAX_WAITS:]
                moved = waits[:-self.MAX_WAITS]
                inst.sync_info = mybir.SyncInfo(on_wait=keep, on_update=list(si.on_update))
                for i in range(0, len(moved), self.MAX_WAITS):
                    out.append(mybir.InstNoOp(
                        name=f"{inst.name}-waitnop-{i}",
                        engine=inst.engine,
                        sync_info=mybir.SyncInfo(
                            on_wait=moved[i:i + self.MAX_WAITS], on_update=[]),
                        bass_nofuse=True,
                    ))
            out.append(inst)
        return out

    def _lower_ordered_insts(self, ordered):
        for bb_name in list(ordered.keys()):
            ordered[bb_name] = self._split_waits(ordered[bb_name])
        return super()._lower_ordered_insts(ordered)

    def _drain_and_barrier(self, tick_clock, wait_clock):
        drain_inst = self.nc.sync.drain()
        wait_clock.add_sem_waits(drain_inst.ins, ScopedClock({None: tick_clock.global_clock}))
        si = drain_inst.ins.sync_info
        if si is not None and len(si.on_wait) > self.MAX_WAITS:
            waits = list(si.on_wait)
            drain_inst.ins.sync_info = mybir.SyncInfo(
                on_wait=waits[: self.MAX_WAITS], on_update=list(si.on_update))
            rest = waits[self.MAX_WAITS:]
            for i in range(0, len(rest), self.MAX_WAITS):
                extra = self.nc.sync.drain()
                extra.ins.sync_info = mybir.SyncInfo(
                    on_wait=rest[i: i + self.MAX_WAITS], on_update=[])
        self.nc.all_engine_barrier()
        assert self.sems is not None
        popped = self.nc._tile_sem_poison_stack.pop()
        assert popped is self._sem_poison
        self.nc.clear_and_free_semaphores(list(self.sems.allocated().values()))
        self.nc.all_engine_barrier()


def build_nc(n_layers=L):
    nc = bass.Bass(num_devices=N_CORES)

    # ---------------- I/O ----------------
    xT = nc.dram_tensor("xT", [128, C, T], F32, kind="ExternalInput")
    posT = nc.dram_tensor("posT", [128, C, T], F32, kind="ExternalInput")
    tokf = nc.dram_tensor("tokf", [128, C], F32, kind="ExternalInput")
    eg = nc.dram_tensor("eg", [128, C], F32, kind="ExternalInput")
    ebi = nc.dram_tensor("ebi", [128, C], F32, kind="ExternalInput")
    lng = nc.dram_tensor("lng", [128, L, 2, C], F32, kind="ExternalInput")
    lnb = nc.dram_tensor("lnb", [128, L, 2, C], F32, kind="ExternalInput")
    qbi = nc.dram_tensor("qbi", [128, L, C], F32, kind="ExternalInput")
    obi = nc.dram_tensor("obi", [128, L, C], F32, kind="ExternalInput")
    f1bi = nc.dram_tensor("f1bi", [128, L, FFC], F32, kind="ExternalInput")
    f2bi = nc.dram_tensor("f2bi", [128, L, C], F32, kind="ExternalInput")
    mbandi = nc.dram_tensor("mbandi", [128, 1024], BF16, kind="ExternalInput")
    mseli = nc.dram_tensor("mseli", [128, 256], U8, kind="ExternalInput")
    hoffi = nc.dram_tensor("hoffi", [1, 4], I32, kind="ExternalInput")
    wq = nc.dram_tensor("wq", [L, D, D], BF16, kind="ExternalInput")
    wk = nc.dram_tensor("wk", [L, D, D], BF16, kind="ExternalInput")
    wv = nc.dram_tensor("wv", [L, D, D], BF16, kind="ExternalInput")
    wo = nc.dram_tensor("wo", [L, D, D], BF16, kind="ExternalInput")
    w1 = nc.dram_tensor("w1", [L, D, FF], BF16, kind="ExternalInput")
    w2 = nc.dram_tensor("w2", [L, FF, D], BF16, kind="ExternalInput")
    out_d = nc.dram_tensor("out", [128, C, T], F32, kind="ExternalOutput")

    agiKs = [nc.dram_tensor(f"agiK{l}", [AGW], BF16) for l in range(n_layers)]
    agoKs = [nc.dram_tensor(f"agoK{l}", [4, AGW], BF16) for l in range(n_layers)]
    agiVs = [nc.dram_tensor(f"agiV{l}", [AGWV], BF16) for l in range(n_layers)]
    agoVs = [nc.dram_tensor(f"agoV{l}", [4, AGWV], BF16) for l in range(n_layers)]

    ln_ctr = [0]

    with contextlib.ExitStack() as ctx:
        tc = ctx.enter_context(TC(nc))
        # ---------------- persistent pools ----------------
        cst = ctx.enter_context(tc.tile_pool(name="cst", bufs=1))
        hpool = ctx.enter_context(tc.tile_pool(name="hp", bufs=1))
        big = ctx.enter_context(tc.tile_pool(name="big", bufs=1))
        sgl = ctx.enter_context(tc.tile_pool(name="sgl", bufs=1))
        scr4 = ctx.enter_context(tc.tile_pool(name="scr4", bufs=4))
        scr2 = ctx.enter_context(tc.tile_pool(name="scr2", bufs=2))
        scr3 = ctx.enter_context(tc.tile_pool(name="scr3", bufs=2))
        wp4 = ctx.enter_context(tc.tile_pool(name="wp4", bufs=7))
        wp1 = ctx.enter_context(tc.tile_pool(name="wp1", bufs=6))
        wp2 = ctx.enter_context(tc.tile_pool(name="wp2", bufs=2))
        ebp = ctx.enter_context(tc.tile_pool(name="ebp", bufs=12))
        edp = ctx.enter_context(tc.tile_pool(name="edp", bufs=4))

        regKA = ctx.enter_context(nc.gpsimd.register("regKA"))
        regKB = ctx.enter_context(nc.gpsimd.register("regKB"))
        regVA = ctx.enter_context(nc.gpsimd.register("regVA"))
        regVB = ctx.enter_context(nc.gpsimd.register("regVB"))

        # ---------------- load constants ----------------
        def cload(nm, shape, dt, src):
            t = cst.tile(shape, dt, tag=nm, name=nm)
            nc.sync.dma_start(out=t, in_=src)
            return t

        tok_s = cload("tok_s", [128, C], F32, tokf[:, :])
        eg_s = cload("eg_s", [128, C], F32, eg[:, :])
        eb_s = cload("eb_s", [128, C], F32, ebi[:, :])
        lng_s = cload("lng_s", [128, L, 2, C], F32, lng[:, :, :, :])
        lnb_s = cload("lnb_s", [128, L, 2, C], F32, lnb[:, :, :, :])
        qb_s = cload("qb_s", [128, L, C], F32, qbi[:, :, :])
        ob_s = cload("ob_s", [128, L, C], F32, obi[:, :, :])
        f1b_s = cload("f1b_s", [128, L, FFC], F32, f1bi[:, :, :])
        f2b_s = cload("f2b_s", [128, L, C], F32, f2bi[:, :, :])
        mband = cload("mband", [128, 1024], BF16, mbandi[:, :])
        msel = cload("msel", [128, 256], U8, mseli[:, :])
        hoff_s = cload("hoff_s", [1, 4], I32, hoffi[:, :])

        onesP = cst.tile([128, 1], F32)
        nc.vector.memset(onesP, 1.0)
        onesPb = cst.tile([128, 1], BF16)
        nc.vector.memset(onesPb, 1.0)
        onesPP = cst.tile([128, 128], F32)
        nc.vector.memset(onesPP, 1.0)
        eps_row = cst.tile([1, 1], F32)
        nc.vector.memset(eps_row, EPS)

        nc.gpsimd.reg_load(regKA, hoff_s[0:1, 0:1])
        nc.gpsimd.reg_load(regKB, hoff_s[0:1, 1:2])
        nc.gpsimd.reg_load(regVA, hoff_s[0:1, 2:3])
        nc.gpsimd.reg_load(regVB, hoff_s[0:1, 3:4])

        # ---------------- persistent tiles (hoisted out of the layer loop) ----
        h = hpool.tile([128, C, T], F32)
        hb = sgl.tile([128, C, T], BF16, tag="hb")
        qT = sgl.tile([128, C, T], BF16, tag="qT")
        kT = sgl.tile([128, C, T], BF16, tag="kT")
        v65o = sgl.tile([128, 4, H, 65], BF16, tag="v65o")
        kTf = big.tile([128, 4, C, T], BF16, tag="kTf")
        v65f = big.tile([128, 4, 4, H, 65], BF16, tag="v65f")
        kThalo = sgl.tile([128, C, 2, 128], BF16, tag="kThalo")
        v65h = sgl.tile([128, 2, H, 65], BF16, tag="v65h")
        cb_sb = big.tile([128, H, T], BF16, tag="cbs")
        ctxT = sgl.tile([128, C, T], BF16, tag="ctxT")
        recip = sgl.tile([1, H, T], F32, tag="recip")
        nc.vector.memset(v65o[:, :, :, 64:65], 1.0)

        # views: eb column space is [hA*A0|A0*A0|A0*A1|A1*A1|hB*B0|B0*B0|B0*B1|B1*B1]
        def half_view(t_, lo, hi):
            return t_.rearrange("p (a b) -> p a b", a=2)[:, :, lo:hi]

        # ---------------- h / embeddings ----------------
        nc.sync.dma_start(out=h, in_=xT[:, :, :])
        for k in range(C):
            pk = scr2.tile([128, T], F32, tag="lnt")
            nc.sync.dma_start(out=pk, in_=posT[:, k, :])
            nc.vector.tensor_add(out=h[:, k, :], in0=h[:, k, :], in1=pk)
            nc.vector.tensor_scalar_add(h[:, k, :], h[:, k, :], tok_s[:, k:k + 1])

        def emit_ln(g_ap, b_ap):
            """post-LN over features (partition axis), h updated in place."""
            ln_ctr[0] += 1
            with tc.tile_pool(name=f"lnps{ln_ctr[0]}", bufs=1, space="PSUM") as lnps, \
                 tc.tile_pool(name=f"lnps2_{ln_ctr[0]}", bufs=2, space="PSUM") as lnps2:
                s1 = lnps.tile([1, T], F32, tag="s1")
                s2 = lnps.tile([1, T], F32, tag="s2")
                for k in range(C):
                    sq = scr2.tile([128, T], BF16, tag="sq")
                    nc.vector.tensor_mul(out=sq, in0=h[:, k, :], in1=h[:, k, :])
                    nc.tensor.matmul(s1, onesP[:, :], h[:, k, :],
                                     start=(k == 0), stop=(k == C - 1))
                    nc.tensor.matmul(s2, onesPb[:, :], sq,
                                     start=(k == 0), stop=(k == C - 1))
                mean = scr4.tile([1, T], F32, tag="row")
                nc.scalar.activation(out=mean, in_=s1, func=AF.Copy, scale=1.0 / D)
                ex2 = scr4.tile([1, T], F32, tag="row")
                nc.scalar.activation(out=ex2, in_=s2, func=AF.Copy, scale=1.0 / D)
                var = scr4.tile([1, T], F32, tag="row")
                nc.gpsimd.tensor_mul(out=var, in0=mean, in1=mean)
                nc.gpsimd.tensor_tensor(out=var, in0=ex2, in1=var, op=ALU.subtract)
                nc.scalar.activation(out=var, in_=var, func=AF.Ln,
                                     bias=eps_row[0:1, 0:1])
                rstd = scr4.tile([1, T], F32, tag="row")
                nc.scalar.activation(out=rstd, in_=var, func=AF.Exp, scale=-0.5)
                mb = lnps2.tile([128, T], F32, tag="mr")
                nc.tensor.matmul(mb, onesPP[0:1, :], mean, start=True, stop=True)
                rb = lnps2.tile([128, T], F32, tag="mr")
                nc.tensor.matmul(rb, onesPP[0:1, :], rstd, start=True, stop=True)
                for k in range(C):
                    d_t = scr2.tile([128, T], F32, tag="lnt")
                    nc.vector.tensor_tensor(out=d_t, in0=h[:, k, :], in1=mb,
                                            op=ALU.subtract)
                    nc.vector.tensor_tensor(out=d_t, in0=d_t, in1=rb, op=ALU.mult)
                    nc.vector.tensor_scalar(out=h[:, k, :], in0=d_t,
                                            scalar1=g_ap[:, k:k + 1],
                                            scalar2=b_ap[:, k:k + 1],
                                            op0=ALU.mult, op1=ALU.add)

        emit_ln(eg_s[:, :], eb_s[:, :])

        # ---------------- layers ----------------
        wk_next = [None]
        for l in range(n_layers):
            agiK, agoK, agiV, agoV = agiKs[l], agoKs[l], agiVs[l], agoVs[l]

            # bf16 cast of h for matmul inputs
            for k in range(C):
                nc.scalar.activation(out=hb[:, k, :], in_=h[:, k, :], func=AF.Copy)

            with tc.tile_pool(name=f"psqkv{l}", bufs=3, space="PSUM") as psq:
                # ---- K projection first, kick off its AllGather ----
                # (wk tiles were prefetched: layer 0 below, layers 1+ during the
                #  previous layer's O-projection section)
                if l == 0:
                    wk_t = []
                    for k in range(C):
                        wt_ = wp4.tile([128, D], BF16, tag="w4")
                        nc.sync.dma_start(out=wt_, in_=wk[l, 128 * k:128 * (k + 1), :])
                        wk_t.append(wt_)
                else:
                    wk_t = wk_next[0]
                for m in range(C):
                    ps = psq.tile([128, T], F32, tag="ps")
                    for k in range(C):
                        nc.tensor.matmul(ps, wk_t[k][:, 128 * m:128 * (m + 1)],
                                         hb[:, k, :], start=(k == 0), stop=(k == C - 1))
                    nc.vector.tensor_copy(out=kT[:, m, :], in_=ps)
                wkt = nc.sync.dma_start(
                    out=agiK[:].rearrange("(p k n) -> p k n", p=128, k=C, n=T),
                    in_=kT[:, :, :])
                ccK = nc.gpsimd.collective_compute(
                    "AllGather", ALU.bypass, replica_groups=GROUPS,
                    ins=[agiK[:]], outs=[agoK[:, :]])
                add_dep_helper(ccK.ins, wkt.ins, reason="AG-K waits kT bounce")

                # ---- V projection, kick off its AllGather ----
                wv_t = []
                for k in range(C):
                    wt_ = wp4.tile([128, D], BF16, tag="w4")
                    nc.sync.dma_start(out=wt_, in_=wv[l, 128 * k:128 * (k + 1), :])
                    wv_t.append(wt_)
                for t in range(4):
                    ps = psq.tile([128, T], F32, tag="ps")
                    ps2 = psq.tile([128, 256], F32, tag="ps2")
                    for k in range(C):
                        nc.tensor.matmul(ps, hb[:, k, 128 * t:128 * (t + 1)],
                                         wv_t[k][:, 0:512], start=(k == 0), stop=(k == C - 1))
                        nc.tensor.matmul(ps2, hb[:, k, 128 * t:128 * (t + 1)],
                                         wv_t[k][:, 512:768], start=(k == 0), stop=(k == C - 1))
                    nc.vector.tensor_copy(
                        out=v65o[:, t, 0:8, 0:64],
                        in_=ps[:, :].rearrange("p (h d) -> p h d", h=8))
                    nc.vector.tensor_copy(
                        out=v65o[:, t, 8:12, 0:64],
                        in_=ps2[:, :].rearrange("p (h d) -> p h d", h=4))
                wvt = nc.sync.dma_start(
                    out=agiV[:].rearrange("(p x) -> p x", p=128, x=4 * H * 65),
                    in_=v65o[:, :, :, :])
                ccV = nc.gpsimd.collective_compute(
                    "AllGather", ALU.bypass, replica_groups=GROUPS,
                    ins=[agiV[:]], outs=[agoV[:, :]])
                add_dep_helper(ccV.ins, wvt.ins, reason="AG-V waits V bounce")

                # ---- Q projection ----
                wq_t = []
                for k in range(C):
                    wt_ = wp4.tile([128, D], BF16, tag="w4")
                    nc.sync.dma_start(out=wt_, in_=wq[l, 128 * k:128 * (k + 1), :])
                    wq_t.append(wt_)
                for m in range(C):
                    ps = psq.tile([128, T], F32, tag="ps")
                    for k in range(C):
                        nc.tensor.matmul(ps, wq_t[k][:, 128 * m:128 * (m + 1)],
                                         hb[:, k, :], start=(k == 0), stop=(k == C - 1))
                    nc.vector.tensor_scalar_add(qT[:, m, :], ps, qb_s[:, l, m:m + 1])

            # ---- band-local scores + exp (overlaps the AllGathers) ----
            ebs = []
            with tc.tile_pool(name=f"psBL{l}", bufs=2, space="PSUM") as psbl:
                for hd_i in range(H):
                    hp, ck = 64 * (hd_i % 2), hd_i // 2
                    sc = psbl.tile([128, 1024], F32, tag="scL")
                    segs = [
                        (128, 256, kT[hp:hp + 64, ck, 0:128], 0, True, False),
                        (384, 128, kT[hp:hp + 64, ck, 128:256], 128, False, True),
                        (640, 256, kT[hp:hp + 64, ck, 256:384], 256, True, False),
                        (896, 128, kT[hp:hp + 64, ck, 384:512], 384, False, True),
                    ]
                    for cs, w, lhs, qs, st, sp in segs:
                        nc.tensor.matmul(sc[:, cs:cs + w], lhs,
                                         qT[hp:hp + 64, ck, qs:qs + w],
                                         start=st, stop=sp, skip_group_check=True)
                    eb_t = ebp.tile([128, 1024], BF16, tag="expB", name=f"eb_{l}_{hd_i}")
                    nc.scalar.activation(out=half_view(eb_t, 128, 512),
                                         in_=half_view(sc, 128, 512),
                                         func=AF.Exp, scale=SCALE)
                    nc.gpsimd.tensor_mul(out=half_view(eb_t, 128, 512),
                                         in0=half_view(eb_t, 128, 512),
                                         in1=half_view(mband, 128, 512))
                    ebs.append(eb_t)

            # ---- fetch gathered kT + halo keys (waits AG-K) ----
            for r in range(4):
                f1 = nc.sync.dma_start(
                    out=kTf[:, r, :, :],
                    in_=agoK[r, :].rearrange("(p k n) -> p k n", p=128, k=C, n=T))
                add_dep_helper(f1.ins, ccK.ins, reason="kTf fetch waits AG-K")
            for s, rk in enumerate([regKA, regKB]):
                fh = nc.gpsimd.dma_start(
                    out=kThalo[:, :, s, :],
                    in_=bass.AP(agoK, rk, [[C * T, 128], [T, C], [1, 128]]))
                add_dep_helper(fh.ins, ccK.ins, reason="kT halo waits AG-K")

            # ---- band halo scores + exp ----
            with tc.tile_pool(name=f"psBH{l}", bufs=2, space="PSUM") as psbh:
                for hd_i in range(H):
                    hp, ck = 64 * (hd_i % 2), hd_i // 2
                    sch = psbh.tile([128, 256], F32, tag="scH")
                    nc.tensor.matmul(sch[:, 0:128], kThalo[hp:hp + 64, ck, 0, :],
                                     qT[hp:hp + 64, ck, 0:128],
                                     start=True, stop=False, skip_group_check=True)
                    nc.tensor.matmul(sch[:, 128:256], kThalo[hp:hp + 64, ck, 1, :],
                                     qT[hp:hp + 64, ck, 256:384],
                                     start=False, stop=True, skip_group_check=True)
                    eb_t = ebs[hd_i]
                    nc.scalar.activation(out=half_view(eb_t, 0, 128),
                                         in_=sch.rearrange("p (a b) -> p a b", a=2),
                                         func=AF.Exp, scale=SCALE)
                    nc.gpsimd.tensor_mul(out=half_view(eb_t, 0, 128),
                                         in0=half_view(eb_t, 0, 128),
                                         in1=half_view(mband, 0, 128))

            # ---- fetch gathered V + halo values (waits AG-V) ----
            for r in range(4):
                f2 = nc.sync.dma_start(
                    out=v65f[:, r, :, :, :],
                    in_=agoV[r, :].rearrange("(p t h e) -> p t h e",
                                             p=128, t=4, h=H, e=65))
                add_dep_helper(f2.ins, ccV.ins, reason="V fetch waits AG-V")
            for s, rv in enumerate([regVA, regVB]):
                fv = nc.gpsimd.dma_start(
                    out=v65h[:, s, :, :],
                    in_=bass.AP(agoV, rv, [[4 * H * 65, 128], [1, H * 65]]))
                add_dep_helper(fv.ins, ccV.ins, reason="V halo waits AG-V")

            # ---- dense attention for the B half: r-outer, AV consumed eagerly ----
            with tc.tile_pool(name=f"psDc{l}", bufs=3, space="PSUM") as psdc:
                for half in range(2):
                    hds = list(range(6 * half, 6 * half + 6))
                    cdps = [psdc.tile([128, 512], F32, tag="ctxd",
                                      name=f"cdp_{l}_{half}_{i}") for i in range(3)]
                    with tc.tile_pool(name=f"psD{l}_{half}", bufs=2, space="PSUM") as psd:
                        for r in range(4):
                            for sub in range(2):
                                shds = hds[3 * sub:3 * sub + 3]
                                eds = []
                                for hd_i in shds:
                                    hp, ck = 64 * (hd_i % 2), hd_i // 2
                                    sd = psd.tile([128, 1024], F32, tag="scD")
                                    for j in range(4):
                                        nc.tensor.matmul(
                                            sd[:, 256 * j:256 * (j + 1)],
                                            kTf[hp:hp + 64, r, ck, 128 * j:128 * (j + 1)],
                                            qT[hp:hp + 64, ck, 256:512],
                                            start=(j % 2 == 0), stop=(j % 2 == 1),
                                            skip_group_check=True)
                                    ed = edp.tile([128, 1024], BF16, tag="expD")
                                    nc.scalar.activation(out=ed, in_=sd, func=AF.Exp,
                                                         scale=SCALE)
                                    eds.append(ed)
                                for i, hd_i in enumerate(shds):
                                    cd = cdps[(hd_i % 6) // 2]
                                    co = 256 * (hd_i % 2)
                                    for j in range(4):
                                        nc.tensor.matmul(
                                            cd[0:65, co:co + 256], v65f[:, r, j, hd_i, :],
                                            eds[i][:, 256 * j:256 * (j + 1)],
                                            start=(r == 0 and j == 0 and hd_i % 2 == 0),
                                            stop=(r == 3 and j == 3 and hd_i % 2 == 1),
                                            skip_group_check=True)
                    # band AV + blend for this half's heads
                    with tc.tile_pool(name=f"psA{l}_{half}", bufs=2, space="PSUM") as psa:
                        for hd_i in hds:
                            cps = psa.tile([128, T], F32, tag="ctx")
                            av = [
                                (0, v65h[:, 0, hd_i, :], 0),
                                (0, v65o[:, 0, hd_i, :], 128),
                                (128, v65o[:, 0, hd_i, :], 256),
                                (128, v65o[:, 1, hd_i, :], 384),
                                (256, v65h[:, 1, hd_i, :], 512),
                                (256, v65o[:, 2, hd_i, :], 640),
                                (384, v65o[:, 2, hd_i, :], 768),
                                (384, v65o[:, 3, hd_i, :], 896),
                            ]
                            eb_t = ebs[hd_i]
                            for i, (ocs, vsrc, ecs) in enumerate(av):
                                nc.tensor.matmul(cps[0:65, ocs:ocs + 128], vsrc,
                                                 eb_t[:, ecs:ecs + 128],
                                                 start=(i == 0), stop=(i == len(av) - 1),
                                                 skip_group_check=True)
                            nc.vector.tensor_copy(out=cb_sb[0:65, hd_i, :],
                                                  in_=cps[0:65, :])
                            cd = cdps[(hd_i % 6) // 2]
                            co = 256 * (hd_i % 2)
                            nc.vector.copy_predicated(
                                cb_sb[0:65, hd_i, 256:512], msel[0:65, :],
                                cd[0:65, co:co + 256])
                    # 1/sum for this half's heads as exp(-ln(sum)) on the
                    # Activation engine (DVE reciprocal is ~3.3us per row and
                    # would serialize the band-AV phase)
                    h0 = 6 * half
                    nc.scalar.activation(out=recip[0:1, h0:h0 + 6, :],
                                         in_=cb_sb[64:65, h0:h0 + 6, :],
                                         func=AF.Ln)
                    nc.scalar.activation(out=recip[0:1, h0:h0 + 6, :],
                                         in_=recip[0:1, h0:h0 + 6, :],
                                         func=AF.Exp, scale=-1.0)

            # ---- normalize ctx, pack to feature-major ctxT ----
            with tc.tile_pool(name=f"psN{l}", bufs=2, space="PSUM") as psn:
                for c in range(C):
                    he, ho = 2 * c, 2 * c + 1
                    rbe = psn.tile([128, T], F32, tag="rps")
                    nc.tensor.matmul(rbe[0:64, :], onesPP[0:1, 0:64], recip[0:1, he, :],
                                     start=True, stop=True)
                    rbo = psn.tile([128, T], F32, tag="rps")
                    nc.tensor.matmul(rbo[0:64, :], onesPP[0:1, 0:64], recip[0:1, ho, :],
                                     start=True, stop=True)
                    nc.vector.tensor_mul(out=ctxT[0:64, c, :],
                                         in0=cb_sb[0:64, he, :], in1=rbe[0:64, :])
                    nc.vector.tensor_mul(out=ctxT[64:128, c, :],
                                         in0=cb_sb[0:64, ho, :], in1=rbo[0:64, :])

            # ---- O projection + residual + LN ----
            with tc.tile_pool(name=f"psO{l}", bufs=2, space="PSUM") as pso:
                wo_t = []
                for k in range(C):
                    wt_ = wp4.tile([128, D], BF16, tag="w4")
                    nc.sync.dma_start(out=wt_, in_=wo[l, 128 * k:128 * (k + 1), :])
                    wo_t.append(wt_)
                if l + 1 < n_layers:
                    wk_next[0] = []
                    for k in range(C):
                        wt_ = wp4.tile([128, D], BF16, tag="w4")
                        nc.sync.dma_start(out=wt_,
                                          in_=wk[l + 1, 128 * k:128 * (k + 1), :])
                        wk_next[0].append(wt_)
                for m in range(C):
                    ps = pso.tile([128, T], F32, tag="ops")
                    for k in range(C):
                        nc.tensor.matmul(ps, wo_t[k][:, 128 * m:128 * (m + 1)],
                                         ctxT[:, k, :], start=(k == 0), stop=(k == C - 1))
                    nc.vector.scalar_tensor_tensor(
                        out=h[:, m, :], in0=ps, scalar=ob_s[:, l, m:m + 1],
                        in1=h[:, m, :], op0=ALU.add, op1=ALU.add)
            emit_ln(lng_s[:, l, 0, :], lnb_s[:, l, 0, :])

            # ---- FFN + residual + LN ----
            for k in range(C):
                nc.scalar.activation(out=hb[:, k, :], in_=h[:, k, :], func=AF.Copy)
            with tc.tile_pool(name=f"psF1{l}", bufs=2, space="PSUM") as psf1, \
                 tc.tile_pool(name=f"psF2{l}", bufs=6, space="PSUM") as psf2:
                f2ps = [psf2.tile([128, T], F32, tag="f2", name=f"f2ps_{l}_{m}") for m in range(C)]
                for quarter in range(4):
                    w1_t = []
                    for k in range(C):
                        wt_ = wp1.tile([128, FF // 4], BF16, tag="w1t")
                        nc.sync.dma_start(
                            out=wt_, in_=w1[l, 128 * k:128 * (k + 1),
                                            quarter * (FF // 4):(quarter + 1) * (FF // 4)])
                        w1_t.append(wt_)
                    for fi in range(FFC // 4):
                        f = quarter * (FFC // 4) + fi
                        w2_t = wp2.tile([128, D], BF16, tag="w2t")
                        nc.sync.dma_start(out=w2_t, in_=w2[l, 128 * f:128 * (f + 1), :])
                        ps1 = psf1.tile([128, T], F32, tag="f1")
                        for k in range(C):
                            nc.tensor.matmul(ps1, w1_t[k][:, 128 * fi:128 * (fi + 1)],
                                             hb[:, k, :], start=(k == 0), stop=(k == C - 1))
                        rl = scr3.tile([128, T], BF16, tag="rl")
                        nc.scalar.activation(out=rl, in_=ps1, func=AF.Relu,
                                             bias=f1b_s[:, l, f:f + 1])
                        for m in range(C):
                            nc.tensor.matmul(f2ps[m], w2_t[:, 128 * m:128 * (m + 1)], rl,
                                             start=(f == 0), stop=(f == FFC - 1))
                for m in range(C):
                    nc.vector.scalar_tensor_tensor(
                        out=h[:, m, :], in0=f2ps[m], scalar=f2b_s[:, l, m:m + 1],
                        in1=h[:, m, :], op0=ALU.add, op1=ALU.add)
            emit_ln(lng_s[:, l, 1, :], lnb_s[:, l, 1, :])

        nc.sync.dma_start(out=out_d[:, :, :], in_=h)
    return nc


def _feat_pack(v):
    """[768] -> [128, 6] feature-major"""
    return np.ascontiguousarray(np.asarray(v, np.float32).reshape(C, 128).T)


def _tok_pack(arr):
    """[512, 768] -> [128, 6, 512] feature-major"""
    a = np.asarray(arr, np.float32).T.reshape(C, 128, T)
    return np.ascontiguousarray(a.transpose(1, 0, 2))


def make_in_maps(inputs):
    lengths = np.asarray(inputs["lengths"]).astype(np.int64)
    window = int(np.asarray(inputs["window"]))
    assert window == WINDOW and np.all(lengths >= S // 2)
    x = np.asarray(inputs["inputs"], np.float32)
    pos = np.asarray(inputs["pos_emb"], np.float32)
    tok = np.asarray(inputs["tok_emb"], np.float32)

    bf = ml_dtypes.bfloat16
    ow = np.asarray(inputs["o_w"], np.float32)
    vb = np.asarray(inputs["v_b"], np.float32)
    ob_eff = np.asarray(inputs["o_b"], np.float32) + np.einsum("ld,lde->le", vb, ow)

    shared = {
        "tokf": _feat_pack(tok[0]),
        "eg": _feat_pack(inputs["emb_ln_g"]),
        "ebi": _feat_pack(inputs["emb_ln_b"]),
        "lng": np.ascontiguousarray(np.stack(
            [np.stack([_feat_pack(np.asarray(inputs["attn_ln_g"])[li]),
                       _feat_pack(np.asarray(inputs["out_ln_g"])[li])], 1)
             for li in range(L)], 1), np.float32),
        "lnb": np.ascontiguousarray(np.stack(
            [np.stack([_feat_pack(np.asarray(inputs["attn_ln_b"])[li]),
                       _feat_pack(np.asarray(inputs["out_ln_b"])[li])], 1)
             for li in range(L)], 1), np.float32),
        "qbi": np.ascontiguousarray(np.stack(
            [_feat_pack(np.asarray(inputs["q_b"])[li]) for li in range(L)], 1), np.float32),
        "obi": np.ascontiguousarray(np.stack(
            [_feat_pack(ob_eff[li]) for li in range(L)], 1), np.float32),
        "f1bi": np.ascontiguousarray(np.stack(
            [np.ascontiguousarray(np.asarray(inputs["ff1_b"], np.float32)[li].reshape(FFC, 128).T)
             for li in range(L)], 1), np.float32),
        "f2bi": np.ascontiguousarray(np.stack(
            [_feat_pack(np.asarray(inputs["ff2_b"])[li]) for li in range(L)], 1), np.float32),
        "wq": np.asarray(inputs["q_w"], np.float32).astype(bf),
        "wk": np.asarray(inputs["k_w"], np.float32).astype(bf),
        "wv": np.asarray(inputs["v_w"], np.float32).astype(bf),
        "wo": np.asarray(inputs["o_w"], np.float32).astype(bf),
        "w1": np.asarray(inputs["ff1_w"], np.float32).astype(bf),
        "w2": np.asarray(inputs["ff2_w"], np.float32).astype(bf),
    }

    io = np.arange(128)
    m_own = (io[:, None] <= io[None, :]).astype(np.float32)      # key off <= q off
    m_prev = (io[:, None] > io[None, :]).astype(np.float32)      # prev-block band
    in_maps = []
    for cidx in range(N_CORES):
        b, q = cidx // 4, cidx % 4
        rows = _rows_for(q)
        lb_ = int(lengths[b])
        mhA = np.zeros((128, 128), np.float32) if q == 0 else m_prev
        # segments: [haloA | b0:own+prev | b1:own | haloB:prev | b2:own+prev | b3:own]
        mb_ = np.concatenate([mhA, m_own, m_prev, m_own,
                              m_prev, m_own, m_prev, m_own], axis=1)
        msel_ = (np.asarray([1024 + 256 * q + cc_ for cc_ in range(256)]) >= lb_)
        gA = 2 * q - 1 if q > 0 else 15
        gB = 7 + 2 * q

        def _owner(g):
            return g // 2 if g < 8 else (g - 8) // 2

        def _lb(g):
            return g % 2 if g < 8 else 2 + (g - 8) % 2

        hoff = []
        for g in (gA, gB):
            hoff.append(_owner(g) * AGW + 128 * _lb(g))          # kT halo in agoK
        for g in (gA, gB):
            hoff.append(_owner(g) * AGWV + _lb(g) * H * 65)      # V halo in agoV
        im = dict(shared)
        im["xT"] = _tok_pack(x[b][rows])
        im["posT"] = _tok_pack(pos[rows])
        im["mbandi"] = mb_.astype(bf)
        im["mseli"] = np.broadcast_to(msel_, (128, 256)).astype(np.uint8).copy()
        im["hoffi"] = np.asarray([hoff], np.int32)
        in_maps.append(im)
    return in_maps


def assemble_output(results):
    out = np.zeros((B, S, D), np.float32)
    for cidx in range(N_CORES):
        b, q = cidx // 4, cidx % 4
        rows = _rows_for(q)
        hc = np.asarray(results[cidx]["out"])          # [128, 6, 512]
        out[b, rows, :] = hc.transpose(1, 0, 2).reshape(D, T).T
    return out


_cached = {}


def kernel(**inputs):
    n_layers = int(inputs.pop("_n_layers", L))
    if n_layers not in _cached:
        _cached[n_layers] = build_nc(n_layers)
    nc = _cached[n_layers]
    in_maps = make_in_maps(inputs)
    res = run_bass_kernel_spmd(nc, in_maps, core_ids=list(range(N_CORES)))
    return assemble_output(res.results)


# revision 17
# speedup vs baseline: 1.0996x; 1.0549x over previous
"""Trainium2 Bass kernel: 4-layer sliding-window causal transformer (B=2, S=2048,
D=768, H=12, FF=3072, window=128) on 8 NeuronCores.

Sharding: 2 batch groups x 4-way sequence split. Core c handles batch b=c//4,
owning 512 tokens: A=[256q,256q+256) and B=[1024+256q,1024+256q+256) (q=c%4).
The B half covers all possibly-invalid rows (lengths >= S/2); those rows get
dense attention over all 2048 keys (matching the reference's uniform -10000
additive mask, which cancels in softmax), merged by a select mask.

Per layer, TWO AllGathers (bf16) within each 4-core group: kT first (starts
right after the K projection) and V second, so band-local attention and the
Q/V projections overlap the collectives, and remote-key score work starts as
soon as kT lands.  Halo key/value blocks are fetched with register-offset DMAs
whose offsets arrive as per-core input data (keeps the SPMD program uniform).

Everything on-chip is feature-major ([d on partitions, tokens free]); matmuls
run in bf16 with f32 accumulation; LayerNorm/softmax math in f32.
"""
import contextlib
import numpy as np
import ml_dtypes

import concourse.bass as bass
import concourse.tile as tile
from concourse import mybir
from concourse.vector_clock import ScopedClock
from concourse.tile_rust import add_dep_helper
from concourse.bass_utils import run_bass_kernel_spmd

F32 = mybir.dt.float32
BF16 = mybir.dt.bfloat16
U8 = mybir.dt.uint8
I32 = mybir.dt.int32
AF = mybir.ActivationFunctionType
ALU = mybir.AluOpType

B, S, D, H, L, FF, WINDOW, HD = 2, 2048, 768, 12, 4, 3072, 128, 64
C = D // 128          # 6 feature chunks
T = 512               # tokens per core
FFC = FF // 128       # 24
AGW = D * T           # 393216 elements in the kT AG payload
AGWV = 4 * H * 65 * 128  # V AG payload: per-partition [t, h, 65] incl. ones column
GROUPS = [[0, 1, 2, 3], [4, 5, 6, 7]]
N_CORES = 8
EPS = 1e-6
SCALE = 1.0 / np.sqrt(HD)


def _rows_for(q):
    return list(range(256 * q, 256 * q + 256)) + list(range(1024 + 256 * q, 1024 + 256 * q + 256))


class TC(tile.TileContext):
    """TileContext workarounds for this toolchain's walrus sync-wait limit:
    at most ONE sync wait per instruction; excess waits move onto injected
    no-fuse NoOps running just before it on the same engine."""
    MAX_WAITS = 1

    def _split_waits(self, insts):
        out = []
        for inst in insts:
            si = getattr(inst, "sync_info", None)
            waits = list(si.on_wait) if si is not None else []
            if len(waits) > self.MAX_WAITS and inst.engine != mybir.EngineType.Unassigned:
                keep = waits[-self.MAX_WAITS:]
                moved = waits[:-self.MAX_WAITS]
                inst.sync_info = mybir.SyncInfo(on_wait=keep, on_update=list(si.on_update))
                for i in range(0, len(moved), self.MAX_WAITS):
                    out.append(mybir.InstNoOp(
                        name=f"{inst.name}-waitnop-{i}",
                        engine=inst.engine,
                        sync_info=mybir.SyncInfo(
                            on_wait=moved[i:i + self.MAX_WAITS], on_update=[]),
                        bass_nofuse=True,
                    ))
            out.append(inst)
        return out

    def _lower_ordered_insts(self, ordered):
        for bb_name in list(ordered.keys()):
            ordered[bb_name] = self._split_waits(ordered[bb_name])
        return super()._lower_ordered_insts(ordered)

    def _drain_and_barrier(self, tick_clock, wait_clock):
        drain_inst = self.nc.sync.drain()
        wait_clock.add_sem_waits(drain_inst.ins, ScopedClock({None: tick_clock.global_clock}))
        si = drain_inst.ins.sync_info
        if si is not None and len(si.on_wait) > self.MAX_WAITS:
            waits = list(si.on_wait)
            drain_inst.ins.sync_info = mybir.SyncInfo(
                on_wait=waits[: self.MAX_WAITS], on_update=list(si.on_update))
            rest = waits[self.MAX_WAITS:]
            for i in range(0, len(rest), self.MAX_WAITS):
                extra = self.nc.sync.drain()
                extra.ins.sync_info = mybir.SyncInfo(
                    on_wait=rest[i: i + self.MAX_WAITS], on_update=[])
        self.nc.all_engine_barrier()
        assert self.sems is not None
        popped = self.nc._tile_sem_poison_stack.pop()
        assert popped is self._sem_poison
        self.nc.clear_and_free_semaphores(list(self.sems.allocated().values()))
        self.nc.all_engine_barrier()


def build_nc(n_layers=L):
    nc = bass.Bass(num_devices=N_CORES)

    # ---------------- I/O ----------------
    xT = nc.dram_tensor("xT", [128, C, T], F32, kind="ExternalInput")
    posT = nc.dram_tensor("posT", [128, C, T], F32, kind="ExternalInput")
    tokf = nc.dram_tensor("tokf", [128, C], F32, kind="ExternalInput")
    eg = nc.dram_tensor("eg", [128, C], F32, kind="ExternalInput")
    ebi = nc.dram_tensor("ebi", [128, C], F32, kind="ExternalInput")
    lng = nc.dram_tensor("lng", [128, L, 2, C], F32, kind="ExternalInput")
    lnb = nc.dram_tensor("lnb", [128, L, 2, C], F32, kind="ExternalInput")
    qbi = nc.dram_tensor("qbi", [128, L, C], F32, kind="ExternalInput")
    obi = nc.dram_tensor("obi", [128, L, C], F32, kind="ExternalInput")
    f1bi = nc.dram_tensor("f1bi", [128, L, FFC], F32, kind="ExternalInput")
    f2bi = nc.dram_tensor("f2bi", [128, L, C], F32, kind="ExternalInput")
    mbandi = nc.dram_tensor("mbandi", [128, 1024], BF16, kind="ExternalInput")
    mseli = nc.dram_tensor("mseli", [128, 256], U8, kind="ExternalInput")
    hoffi = nc.dram_tensor("hoffi", [1, 4], I32, kind="ExternalInput")
    wq = nc.dram_tensor("wq", [L, D, D], BF16, kind="ExternalInput")
    wk = nc.dram_tensor("wk", [L, D, D], BF16, kind="ExternalInput")
    wv = nc.dram_tensor("wv", [L, D, D], BF16, kind="ExternalInput")
    wo = nc.dram_tensor("wo", [L, D, D], BF16, kind="ExternalInput")
    w1 = nc.dram_tensor("w1", [L, D, FF], BF16, kind="ExternalInput")
    w2 = nc.dram_tensor("w2", [L, FF, D], BF16, kind="ExternalInput")
    out_d = nc.dram_tensor("out", [128, C, T], F32, kind="ExternalOutput")

    agiKs = [nc.dram_tensor(f"agiK{l}", [AGW], BF16) for l in range(n_layers)]
    agoKs = [nc.dram_tensor(f"agoK{l}", [4, AGW], BF16) for l in range(n_layers)]
    agiVs = [nc.dram_tensor(f"agiV{l}", [AGWV], BF16) for l in range(n_layers)]
    agoVs = [nc.dram_tensor(f"agoV{l}", [4, AGWV], BF16) for l in range(n_layers)]

    ln_ctr = [0]

    with contextlib.ExitStack() as ctx:
        tc = ctx.enter_context(TC(nc))
        # ---------------- persistent pools ----------------
        cst = ctx.enter_context(tc.tile_pool(name="cst", bufs=1))
        hpool = ctx.enter_context(tc.tile_pool(name="hp", bufs=1))
        big = ctx.enter_context(tc.tile_pool(name="big", bufs=1))
        sgl = ctx.enter_context(tc.tile_pool(name="sgl", bufs=1))
        scr4 = ctx.enter_context(tc.tile_pool(name="scr4", bufs=4))
        scr2 = ctx.enter_context(tc.tile_pool(name="scr2", bufs=2))
        scr3 = ctx.enter_context(tc.tile_pool(name="scr3", bufs=2))
        wp4 = ctx.enter_context(tc.tile_pool(name="wp4", bufs=7))
        wp1 = ctx.enter_context(tc.tile_pool(name="wp1", bufs=6))
        wp2 = ctx.enter_context(tc.tile_pool(name="wp2", bufs=2))
        ebp = ctx.enter_context(tc.tile_pool(name="ebp", bufs=12))
        edp = ctx.enter_context(tc.tile_pool(name="edp", bufs=4))

        regKA = ctx.enter_context(nc.gpsimd.register("regKA"))
        regKB = ctx.enter_context(nc.gpsimd.register("regKB"))
        regVA = ctx.enter_context(nc.gpsimd.register("regVA"))
        regVB = ctx.enter_context(nc.gpsimd.register("regVB"))

        # ---------------- load constants ----------------
        def cload(nm, shape, dt, src):
            t = cst.tile(shape, dt, tag=nm, name=nm)
            nc.sync.dma_start(out=t, in_=src)
            return t

        tok_s = cload("tok_s", [128, C], F32, tokf[:, :])
        eg_s = cload("eg_s", [128, C], F32, eg[:, :])
        eb_s = cload("eb_s", [128, C], F32, ebi[:, :])
        lng_s = cload("lng_s", [128, L, 2, C], F32, lng[:, :, :, :])
        lnb_s = cload("lnb_s", [128, L, 2, C], F32, lnb[:, :, :, :])
        qb_s = cload("qb_s", [128, L, C], F32, qbi[:, :, :])
        ob_s = cload("ob_s", [128, L, C], F32, obi[:, :, :])
        f1b_s = cload("f1b_s", [128, L, FFC], F32, f1bi[:, :, :])
        f2b_s = cload("f2b_s", [128, L, C], F32, f2bi[:, :, :])
        mband = cload("mband", [128, 1024], BF16, mbandi[:, :])
        msel = cload("msel", [128, 256], U8, mseli[:, :])
        hoff_s = cload("hoff_s", [1, 4], I32, hoffi[:, :])

        onesP = cst.tile([128, 1], F32)
        nc.vector.memset(onesP, 1.0)
        onesPb = cst.tile([128, 1], BF16)
        nc.vector.memset(onesPb, 1.0)
        onesPP = cst.tile([128, 128], F32)
        nc.vector.memset(onesPP, 1.0)
        eps_row = cst.tile([1, 1], F32)
        nc.vector.memset(eps_row, EPS)

        nc.gpsimd.reg_load(regKA, hoff_s[0:1, 0:1])
        nc.gpsimd.reg_load(regKB, hoff_s[0:1, 1:2])
        nc.gpsimd.reg_load(regVA, hoff_s[0:1, 2:3])
        nc.gpsimd.reg_load(regVB, hoff_s[0:1, 3:4])

        # ---------------- persistent tiles (hoisted out of the layer loop) ----
        h = hpool.tile([128, C, T], F32)
        hb = sgl.tile([128, C, T], BF16, tag="hb")
        qT = sgl.tile([128, C, T], BF16, tag="qT")
        kT = sgl.tile([128, C, T], BF16, tag="kT")
        v65o = sgl.tile([128, 4, H, 65], BF16, tag="v65o")
        kTf = big.tile([128, 4, C, T], BF16, tag="kTf")
        v65f = big.tile([128, 4, 4, H, 65], BF16, tag="v65f")
        kThalo = sgl.tile([128, C, 2, 128], BF16, tag="kThalo")
        v65h = sgl.tile([128, 2, H, 65], BF16, tag="v65h")
        cb_sb = big.tile([128, H, T], BF16, tag="cbs")
        ctxT = sgl.tile([128, C, T], BF16, tag="ctxT")
        recip = sgl.tile([1, H, T], F32, tag="recip")
        nc.vector.memset(v65o[:, :, :, 64:65], 1.0)

        # views: eb column space is [hA*A0|A0*A0|A0*A1|A1*A1|hB*B0|B0*B0|B0*B1|B1*B1]
        def half_view(t_, lo, hi):
            return t_.rearrange("p (a b) -> p a b", a=2)[:, :, lo:hi]

        # ---------------- h / embeddings ----------------
        nc.sync.dma_start(out=h, in_=xT[:, :, :])
        for k in range(C):
            pk = scr2.tile([128, T], F32, tag="lnt")
            nc.sync.dma_start(out=pk, in_=posT[:, k, :])
            nc.vector.tensor_add(out=h[:, k, :], in0=h[:, k, :], in1=pk)
            nc.vector.tensor_scalar_add(h[:, k, :], h[:, k, :], tok_s[:, k:k + 1])

        def emit_ln(g_ap, b_ap):
            """post-LN over features (partition axis), h updated in place."""
            ln_ctr[0] += 1
            with tc.tile_pool(name=f"lnps{ln_ctr[0]}", bufs=1, space="PSUM") as lnps, \
                 tc.tile_pool(name=f"lnps2_{ln_ctr[0]}", bufs=2, space="PSUM") as lnps2:
                s1 = lnps.tile([1, T], F32, tag="s1")
                s2 = lnps.tile([1, T], F32, tag="s2")
                for k in range(C):
                    sq = scr2.tile([128, T], BF16, tag="sq")
                    nc.gpsimd.tensor_mul(out=sq, in0=h[:, k, :], in1=h[:, k, :])
                    nc.tensor.matmul(s1, onesP[:, :], h[:, k, :],
                                     start=(k == 0), stop=(k == C - 1))
                    nc.tensor.matmul(s2, onesPb[:, :], sq,
                                     start=(k == 0), stop=(k == C - 1))
                mean = scr4.tile([1, T], F32, tag="row")
                nc.scalar.activation(out=mean, in_=s1, func=AF.Copy, scale=1.0 / D)
                ex2 = scr4.tile([1, T], F32, tag="row")
                nc.scalar.activation(out=ex2, in_=s2, func=AF.Copy, scale=1.0 / D)
                var = scr4.tile([1, T], F32, tag="row")
                nc.gpsimd.tensor_mul(out=var, in0=mean, in1=mean)
                nc.gpsimd.tensor_tensor(out=var, in0=ex2, in1=var, op=ALU.subtract)
                nc.scalar.activation(out=var, in_=var, func=AF.Ln,
                                     bias=eps_row[0:1, 0:1])
                rstd = scr4.tile([1, T], F32, tag="row")
                nc.scalar.activation(out=rstd, in_=var, func=AF.Exp, scale=-0.5)
                mb = lnps2.tile([128, T], F32, tag="mr")
                nc.tensor.matmul(mb, onesPP[0:1, :], mean, start=True, stop=True)
                rb = lnps2.tile([128, T], F32, tag="mr")
                nc.tensor.matmul(rb, onesPP[0:1, :], rstd, start=True, stop=True)
                for k in range(C):
                    d_t = scr2.tile([128, T], F32, tag="lnt")
                    nc.vector.tensor_tensor(out=d_t, in0=h[:, k, :], in1=mb,
                                            op=ALU.subtract)
                    nc.vector.tensor_tensor(out=d_t, in0=d_t, in1=rb, op=ALU.mult)
                    nc.vector.tensor_scalar(out=h[:, k, :], in0=d_t,
                                            scalar1=g_ap[:, k:k + 1],
                                            scalar2=b_ap[:, k:k + 1],
                                            op0=ALU.mult, op1=ALU.add)

        emit_ln(eg_s[:, :], eb_s[:, :])

        # ---------------- layers ----------------
        wk_next = [None]
        for l in range(n_layers):
            agiK, agoK, agiV, agoV = agiKs[l], agoKs[l], agiVs[l], agoVs[l]

            # bf16 cast of h for matmul inputs
            for k in range(C):
                nc.scalar.activation(out=hb[:, k, :], in_=h[:, k, :], func=AF.Copy)

            with tc.tile_pool(name=f"psqkv{l}", bufs=3, space="PSUM") as psq:
                # ---- K projection first, kick off its AllGather ----
                # (wk tiles were prefetched: layer 0 below, layers 1+ during the
                #  previous layer's O-projection section)
                if l == 0:
                    wk_t = []
                    for k in range(C):
                        wt_ = wp4.tile([128, D], BF16, tag="w4")
                        nc.sync.dma_start(out=wt_, in_=wk[l, 128 * k:128 * (k + 1), :])
                        wk_t.append(wt_)
                else:
                    wk_t = wk_next[0]
                for m in range(C):
                    ps = psq.tile([128, T], F32, tag="ps")
                    for k in range(C):
                        nc.tensor.matmul(ps, wk_t[k][:, 128 * m:128 * (m + 1)],
                                         hb[:, k, :], start=(k == 0), stop=(k == C - 1))
                    nc.vector.tensor_copy(out=kT[:, m, :], in_=ps)
                wkt = nc.sync.dma_start(
                    out=agiK[:].rearrange("(p k n) -> p k n", p=128, k=C, n=T),
                    in_=kT[:, :, :])
                ccK = nc.gpsimd.collective_compute(
                    "AllGather", ALU.bypass, replica_groups=GROUPS,
                    ins=[agiK[:]], outs=[agoK[:, :]])
                add_dep_helper(ccK.ins, wkt.ins, reason="AG-K waits kT bounce")

                # ---- V projection, kick off its AllGather ----
                wv_t = []
                for k in range(C):
                    wt_ = wp4.tile([128, D], BF16, tag="w4")
                    nc.sync.dma_start(out=wt_, in_=wv[l, 128 * k:128 * (k + 1), :])
                    wv_t.append(wt_)
                for t in range(4):
                    ps = psq.tile([128, T], F32, tag="ps")
                    ps2 = psq.tile([128, 256], F32, tag="ps2")
                    for k in range(C):
                        nc.tensor.matmul(ps, hb[:, k, 128 * t:128 * (t + 1)],
                                         wv_t[k][:, 0:512], start=(k == 0), stop=(k == C - 1))
                        nc.tensor.matmul(ps2, hb[:, k, 128 * t:128 * (t + 1)],
                                         wv_t[k][:, 512:768], start=(k == 0), stop=(k == C - 1))
                    nc.vector.tensor_copy(
                        out=v65o[:, t, 0:8, 0:64],
                        in_=ps[:, :].rearrange("p (h d) -> p h d", h=8))
                    nc.vector.tensor_copy(
                        out=v65o[:, t, 8:12, 0:64],
                        in_=ps2[:, :].rearrange("p (h d) -> p h d", h=4))
                wvt = nc.sync.dma_start(
                    out=agiV[:].rearrange("(p x) -> p x", p=128, x=4 * H * 65),
                    in_=v65o[:, :, :, :])
                ccV = nc.gpsimd.collective_compute(
                    "AllGather", ALU.bypass, replica_groups=GROUPS,
                    ins=[agiV[:]], outs=[agoV[:, :]])
                add_dep_helper(ccV.ins, wvt.ins, reason="AG-V waits V bounce")

                # ---- Q projection ----
                wq_t = []
                for k in range(C):
                    wt_ = wp4.tile([128, D], BF16, tag="w4")
                    nc.sync.dma_start(out=wt_, in_=wq[l, 128 * k:128 * (k + 1), :])
                    wq_t.append(wt_)
                for m in range(C):
                    ps = psq.tile([128, T], F32, tag="ps")
                    for k in range(C):
                        nc.tensor.matmul(ps, wq_t[k][:, 128 * m:128 * (m + 1)],
                                         hb[:, k, :], start=(k == 0), stop=(k == C - 1))
                    nc.vector.tensor_scalar_add(qT[:, m, :], ps, qb_s[:, l, m:m + 1])

            # ---- band-local scores + exp (overlaps the AllGathers) ----
            ebs = []
            with tc.tile_pool(name=f"psBL{l}", bufs=2, space="PSUM") as psbl:
                for hd_i in range(H):
                    hp, ck = 64 * (hd_i % 2), hd_i // 2
                    sc = psbl.tile([128, 1024], F32, tag="scL")
                    segs = [
                        (128, 256, kT[hp:hp + 64, ck, 0:128], 0, True, False),
                        (384, 128, kT[hp:hp + 64, ck, 128:256], 128, False, True),
                        (640, 256, kT[hp:hp + 64, ck, 256:384], 256, True, False),
                        (896, 128, kT[hp:hp + 64, ck, 384:512], 384, False, True),
                    ]
                    for cs, w, lhs, qs, st, sp in segs:
                        nc.tensor.matmul(sc[:, cs:cs + w], lhs,
                                         qT[hp:hp + 64, ck, qs:qs + w],
                                         start=st, stop=sp, skip_group_check=True)
                    eb_t = ebp.tile([128, 1024], BF16, tag="expB", name=f"eb_{l}_{hd_i}")
                    nc.scalar.activation(out=half_view(eb_t, 128, 512),
                                         in_=half_view(sc, 128, 512),
                                         func=AF.Exp, scale=SCALE)
                    nc.gpsimd.tensor_mul(out=half_view(eb_t, 128, 512),
                                         in0=half_view(eb_t, 128, 512),
                                         in1=half_view(mband, 128, 512))
                    ebs.append(eb_t)

            # ---- fetch gathered kT + halo keys (waits AG-K) ----
            for r in range(4):
                f1 = nc.sync.dma_start(
                    out=kTf[:, r, :, :],
                    in_=agoK[r, :].rearrange("(p k n) -> p k n", p=128, k=C, n=T))
                add_dep_helper(f1.ins, ccK.ins, reason="kTf fetch waits AG-K")
            for s, rk in enumerate([regKA, regKB]):
                fh = nc.gpsimd.dma_start(
                    out=kThalo[:, :, s, :],
                    in_=bass.AP(agoK, rk, [[C * T, 128], [T, C], [1, 128]]))
                add_dep_helper(fh.ins, ccK.ins, reason="kT halo waits AG-K")

            # ---- band halo scores + exp ----
            with tc.tile_pool(name=f"psBH{l}", bufs=2, space="PSUM") as psbh:
                for hd_i in range(H):
                    hp, ck = 64 * (hd_i % 2), hd_i // 2
                    sch = psbh.tile([128, 256], F32, tag="scH")
                    nc.tensor.matmul(sch[:, 0:128], kThalo[hp:hp + 64, ck, 0, :],
                                     qT[hp:hp + 64, ck, 0:128],
                                     start=True, stop=False, skip_group_check=True)
                    nc.tensor.matmul(sch[:, 128:256], kThalo[hp:hp + 64, ck, 1, :],
                                     qT[hp:hp + 64, ck, 256:384],
                                     start=False, stop=True, skip_group_check=True)
                    eb_t = ebs[hd_i]
                    nc.scalar.activation(out=half_view(eb_t, 0, 128),
                                         in_=sch.rearrange("p (a b) -> p a b", a=2),
                                         func=AF.Exp, scale=SCALE)
                    nc.gpsimd.tensor_mul(out=half_view(eb_t, 0, 128),
                                         in0=half_view(eb_t, 0, 128),
                                         in1=half_view(mband, 0, 128))

            # ---- fetch gathered V + halo values (waits AG-V) ----
            for r in range(4):
                f2 = nc.sync.dma_start(
                    out=v65f[:, r, :, :, :],
                    in_=agoV[r, :].rearrange("(p t h e) -> p t h e",
                                             p=128, t=4, h=H, e=65))
                add_dep_helper(f2.ins, ccV.ins, reason="V fetch waits AG-V")
            for s, rv in enumerate([regVA, regVB]):
                fv = nc.gpsimd.dma_start(
                    out=v65h[:, s, :, :],
                    in_=bass.AP(agoV, rv, [[4 * H * 65, 128], [1, H * 65]]))
                add_dep_helper(fv.ins, ccV.ins, reason="V halo waits AG-V")

            # ---- dense attention for the B half: r-outer, AV consumed eagerly ----
            with tc.tile_pool(name=f"psDc{l}", bufs=3, space="PSUM") as psdc:
                for half in range(2):
                    hds = list(range(6 * half, 6 * half + 6))
                    cdps = [psdc.tile([128, 512], F32, tag="ctxd",
                                      name=f"cdp_{l}_{half}_{i}") for i in range(3)]
                    def emit_band_av(hd_i, psa):
                        cps = psa.tile([128, T], F32, tag="ctx")
                        av = [
                            (0, v65h[:, 0, hd_i, :], 0),
                            (0, v65o[:, 0, hd_i, :], 128),
                            (128, v65o[:, 0, hd_i, :], 256),
                            (128, v65o[:, 1, hd_i, :], 384),
                            (256, v65h[:, 1, hd_i, :], 512),
                            (256, v65o[:, 2, hd_i, :], 640),
                            (384, v65o[:, 2, hd_i, :], 768),
                            (384, v65o[:, 3, hd_i, :], 896),
                        ]
                        eb_t = ebs[hd_i]
                        for i, (ocs, vsrc, ecs) in enumerate(av):
                            nc.tensor.matmul(cps[0:65, ocs:ocs + 128], vsrc,
                                             eb_t[:, ecs:ecs + 128],
                                             start=(i == 0), stop=(i == len(av) - 1),
                                             skip_group_check=True)
                        nc.vector.tensor_copy(out=cb_sb[0:65, hd_i, :],
                                              in_=cps[0:65, :])

                    with tc.tile_pool(name=f"psD{l}_{half}", bufs=2, space="PSUM") as psd, \
                         tc.tile_pool(name=f"psA{l}_{half}", bufs=1, space="PSUM") as psa:
                        for r in range(4):
                            for sub in range(2):
                                shds = hds[3 * sub:3 * sub + 3]
                                eds = []
                                for hd_i in shds:
                                    hp, ck = 64 * (hd_i % 2), hd_i // 2
                                    sd = psd.tile([128, 1024], F32, tag="scD")
                                    for j in range(4):
                                        nc.tensor.matmul(
                                            sd[:, 256 * j:256 * (j + 1)],
                                            kTf[hp:hp + 64, r, ck, 128 * j:128 * (j + 1)],
                                            qT[hp:hp + 64, ck, 256:512],
                                            start=(j % 2 == 0), stop=(j % 2 == 1),
                                            skip_group_check=True)
                                    ed = edp.tile([128, 1024], BF16, tag="expD")
                                    nc.scalar.activation(out=ed, in_=sd, func=AF.Exp,
                                                         scale=SCALE)
                                    eds.append(ed)
                                for i, hd_i in enumerate(shds):
                                    cd = cdps[(hd_i % 6) // 2]
                                    co = 256 * (hd_i % 2)
                                    for j in range(4):
                                        nc.tensor.matmul(
                                            cd[0:65, co:co + 256], v65f[:, r, j, hd_i, :],
                                            eds[i][:, 256 * j:256 * (j + 1)],
                                            start=(r == 0 and j == 0 and hd_i % 2 == 0),
                                            stop=(r == 3 and j == 3 and hd_i % 2 == 1),
                                            skip_group_check=True)
                                # fill Act-bound exp latency with band-AV work
                                if r >= 1:
                                    emit_band_av(hds[2 * (r - 1) + sub], psa)
                        for hd_i in hds:
                            cd = cdps[(hd_i % 6) // 2]
                            co = 256 * (hd_i % 2)
                            nc.vector.copy_predicated(
                                cb_sb[0:65, hd_i, 256:512], msel[0:65, :],
                                cd[0:65, co:co + 256])
                    # 1/sum for this half's heads as exp(-ln(sum)) on the
                    # Activation engine (DVE reciprocal is ~3.3us per row and
                    # would serialize the band-AV phase)
                    h0 = 6 * half
                    nc.scalar.activation(out=recip[0:1, h0:h0 + 6, :],
                                         in_=cb_sb[64:65, h0:h0 + 6, :],
                                         func=AF.Ln)
                    nc.scalar.activation(out=recip[0:1, h0:h0 + 6, :],
                                         in_=recip[0:1, h0:h0 + 6, :],
                                         func=AF.Exp, scale=-1.0)

            # ---- normalize ctx, pack to feature-major ctxT ----
            with tc.tile_pool(name=f"psN{l}", bufs=2, space="PSUM") as psn:
                for c in range(C):
                    he, ho = 2 * c, 2 * c + 1
                    rbe = psn.tile([128, T], F32, tag="rps")
                    nc.tensor.matmul(rbe[0:64, :], onesPP[0:1, 0:64], recip[0:1, he, :],
                                     start=True, stop=True)
                    rbo = psn.tile([128, T], F32, tag="rps")
                    nc.tensor.matmul(rbo[0:64, :], onesPP[0:1, 0:64], recip[0:1, ho, :],
                                     start=True, stop=True)
                    nc.vector.tensor_mul(out=ctxT[0:64, c, :],
                                         in0=cb_sb[0:64, he, :], in1=rbe[0:64, :])
                    nc.vector.tensor_mul(out=ctxT[64:128, c, :],
                                         in0=cb_sb[0:64, ho, :], in1=rbo[0:64, :])

            # ---- O projection + residual + LN ----
            with tc.tile_pool(name=f"psO{l}", bufs=2, space="PSUM") as pso:
                wo_t = []
                for k in range(C):
                    wt_ = wp4.tile([128, D], BF16, tag="w4")
                    nc.sync.dma_start(out=wt_, in_=wo[l, 128 * k:128 * (k + 1), :])
                    wo_t.append(wt_)
                if l + 1 < n_layers:
                    wk_next[0] = []
                    for k in range(C):
                        wt_ = wp4.tile([128, D], BF16, tag="w4")
                        nc.sync.dma_start(out=wt_,
                                          in_=wk[l + 1, 128 * k:128 * (k + 1), :])
                        wk_next[0].append(wt_)
                for m in range(C):
                    ps = pso.tile([128, T], F32, tag="ops")
                    for k in range(C):
                        nc.tensor.matmul(ps, wo_t[k][:, 128 * m:128 * (m + 1)],
                                         ctxT[:, k, :], start=(k == 0), stop=(k == C - 1))
                    nc.vector.scalar_tensor_tensor(
                        out=h[:, m, :], in0=ps, scalar=ob_s[:, l, m:m + 1],
                        in1=h[:, m, :], op0=ALU.add, op1=ALU.add)
            emit_ln(lng_s[:, l, 0, :], lnb_s[:, l, 0, :])

            # ---- FFN + residual + LN ----
            for k in range(C):
                nc.scalar.activation(out=hb[:, k, :], in_=h[:, k, :], func=AF.Copy)
            with tc.tile_pool(name=f"psF1{l}", bufs=2, space="PSUM") as psf1, \
                 tc.tile_pool(name=f"psF2{l}", bufs=6, space="PSUM") as psf2:
                f2ps = [psf2.tile([128, T], F32, tag="f2", name=f"f2ps_{l}_{m}") for m in range(C)]
                for quarter in range(4):
                    w1_t = []
                    for k in range(C):
                        wt_ = wp1.tile([128, FF // 4], BF16, tag="w1t")
                        nc.sync.dma_start(
                            out=wt_, in_=w1[l, 128 * k:128 * (k + 1),
                                            quarter * (FF // 4):(quarter + 1) * (FF // 4)])
                        w1_t.append(wt_)
                    for fi in range(FFC // 4):
                        f = quarter * (FFC // 4) + fi
                        w2_t = wp2.tile([128, D], BF16, tag="w2t")
                        nc.sync.dma_start(out=w2_t, in_=w2[l, 128 * f:128 * (f + 1), :])
                        ps1 = psf1.tile([128, T], F32, tag="f1")
                        for k in range(C):
                            nc.tensor.matmul(ps1, w1_t[k][:, 128 * fi:128 * (fi + 1)],
                                             hb[:, k, :], start=(k == 0), stop=(k == C - 1))
                        rl = scr3.tile([128, T], BF16, tag="rl")
                        nc.vector.tensor_scalar(out=rl, in0=ps1,
                                                scalar1=f1b_s[:, l, f:f + 1],
                                                scalar2=0.0,
                                                op0=ALU.add, op1=ALU.max)
                        for m in range(C):
                            nc.tensor.matmul(f2ps[m], w2_t[:, 128 * m:128 * (m + 1)], rl,
                                             start=(f == 0), stop=(f == FFC - 1))
                for m in range(C):
                    nc.vector.scalar_tensor_tensor(
                        out=h[:, m, :], in0=f2ps[m], scalar=f2b_s[:, l, m:m + 1],
                        in1=h[:, m, :], op0=ALU.add, op1=ALU.add)
            emit_ln(lng_s[:, l, 1, :], lnb_s[:, l, 1, :])

        nc.sync.dma_start(out=out_d[:, :, :], in_=h)
    return nc


def _feat_pack(v):
    """[768] -> [128, 6] feature-major"""
    return np.ascontiguousarray(np.asarray(v, np.float32).reshape(C, 128).T)


def _tok_pack(arr):
    """[512, 768] -> [128, 6, 512] feature-major"""
    a = np.asarray(arr, np.float32).T.reshape(C, 128, T)
    return np.ascontiguousarray(a.transpose(1, 0, 2))


def make_in_maps(inputs):
    lengths = np.asarray(inputs["lengths"]).astype(np.int64)
    window = int(np.asarray(inputs["window"]))
    assert window == WINDOW and np.all(lengths >= S // 2)
    x = np.asarray(inputs["inputs"], np.float32)
    pos = np.asarray(inputs["pos_emb"], np.float32)
    tok = np.asarray(inputs["tok_emb"], np.float32)

    bf = ml_dtypes.bfloat16
    ow = np.asarray(inputs["o_w"], np.float32)
    vb = np.asarray(inputs["v_b"], np.float32)
    ob_eff = np.asarray(inputs["o_b"], np.float32) + np.einsum("ld,lde->le", vb, ow)

    shared = {
        "tokf": _feat_pack(tok[0]),
        "eg": _feat_pack(inputs["emb_ln_g"]),
        "ebi": _feat_pack(inputs["emb_ln_b"]),
        "lng": np.ascontiguousarray(np.stack(
            [np.stack([_feat_pack(np.asarray(inputs["attn_ln_g"])[li]),
                       _feat_pack(np.asarray(inputs["out_ln_g"])[li])], 1)
             for li in range(L)], 1), np.float32),
        "lnb": np.ascontiguousarray(np.stack(
            [np.stack([_feat_pack(np.asarray(inputs["attn_ln_b"])[li]),
                       _feat_pack(np.asarray(inputs["out_ln_b"])[li])], 1)
             for li in range(L)], 1), np.float32),
        "qbi": np.ascontiguousarray(np.stack(
            [_feat_pack(np.asarray(inputs["q_b"])[li]) for li in range(L)], 1), np.float32),
        "obi": np.ascontiguousarray(np.stack(
            [_feat_pack(ob_eff[li]) for li in range(L)], 1), np.float32),
        "f1bi": np.ascontiguousarray(np.stack(
            [np.ascontiguousarray(np.asarray(inputs["ff1_b"], np.float32)[li].reshape(FFC, 128).T)
             for li in range(L)], 1), np.float32),
        "f2bi": np.ascontiguousarray(np.stack(
            [_feat_pack(np.asarray(inputs["ff2_b"])[li]) for li in range(L)], 1), np.float32),
        "wq": np.asarray(inputs["q_w"], np.float32).astype(bf),
        "wk": np.asarray(inputs["k_w"], np.float32).astype(bf),
        "wv": np.asarray(inputs["v_w"], np.float32).astype(bf),
        "wo": np.asarray(inputs["o_w"], np.float32).astype(bf),
        "w1": np.asarray(inputs["ff1_w"], np.float32).astype(bf),
        "w2": np.asarray(inputs["ff2_w"], np.float32).astype(bf),
    }

    io = np.arange(128)
    m_own = (io[:, None] <= io[None, :]).astype(np.float32)      # key off <= q off
    m_prev = (io[:, None] > io[None, :]).astype(np.float32)      # prev-block band
    in_maps = []
    for cidx in range(N_CORES):
        b, q = cidx // 4, cidx % 4
        rows = _rows_for(q)
        lb_ = int(lengths[b])
        mhA = np.zeros((128, 128), np.float32) if q == 0 else m_prev
        # segments: [haloA | b0:own+prev | b1:own | haloB:prev | b2:own+prev | b3:own]
        mb_ = np.concatenate([mhA, m_own, m_prev, m_own,
                              m_prev, m_own, m_prev, m_own], axis=1)
        msel_ = (np.asarray([1024 + 256 * q + cc_ for cc_ in range(256)]) >= lb_)
        gA = 2 * q - 1 if q > 0 else 15
        gB = 7 + 2 * q

        def _owner(g):
            return g // 2 if g < 8 else (g - 8) // 2

        def _lb(g):
            return g % 2 if g < 8 else 2 + (g - 8) % 2

        hoff = []
        for g in (gA, gB):
            hoff.append(_owner(g) * AGW + 128 * _lb(g))          # kT halo in agoK
        for g in (gA, gB):
            hoff.append(_owner(g) * AGWV + _lb(g) * H * 65)      # V halo in agoV
        im = dict(shared)
        im["xT"] = _tok_pack(x[b][rows])
        im["posT"] = _tok_pack(pos[rows])
        im["mbandi"] = mb_.astype(bf)
        im["mseli"] = np.broadcast_to(msel_, (128, 256)).astype(np.uint8).copy()
        im["hoffi"] = np.asarray([hoff], np.int32)
        in_maps.append(im)
    return in_maps


def assemble_output(results):
    out = np.zeros((B, S, D), np.float32)
    for cidx in range(N_CORES):
        b, q = cidx // 4, cidx % 4
        rows = _rows_for(q)
        hc = np.asarray(results[cidx]["out"])          # [128, 6, 512]
        out[b, rows, :] = hc.transpose(1, 0, 2).reshape(D, T).T
    return out


_cached = {}


def kernel(**inputs):
    n_layers = int(inputs.pop("_n_layers", L))
    if n_layers not in _cached:
        _cached[n_layers] = build_nc(n_layers)
    nc = _cached[n_layers]
    in_maps = make_in_maps(inputs)
    res = run_bass_kernel_spmd(nc, in_maps, core_ids=list(range(N_CORES)))
    return assemble_output(res.results)
